# revision 16
# baseline (speedup 1.0000x reference)
"""Trainium2 Bass kernel for nn_AtomsGPT (GPT-2-style dense transformer).

B=4, T=1024, D=1024, H=16 heads, L=8 layers, V=50257, tied LM head.

Sharding (8 NeuronCores):
- Token-data-parallel trunk: core c owns batch c//2, pair-rank r=c%2.
  Rank r takes the even (r=0) / odd (r=1) 128-position tiles of the
  sequence, interleaved for causal-attention load balance.
- Per layer the pair exchanges LN1 outputs via a 2-rank AllGather.  The
  local K/V matmuls run from the local xn while the AllGather is in
  flight; the remote half is recovered bit-exactly as
  (bank0+bank1) - xn_local in fp32, keeping the program rank-agnostic
  (k_all/v_all banks are core-relative: bank0=local, bank1=remote).
- Attention processes head PAIRS: the two 64-wide score matmuls use
  disjoint PE row groups (tile_position auto-derived) and run
  concurrently.  Causal masks are per-core input data (local diagonal =
  triangular for both ranks; remote diagonal = all-ones / all-zeros).
- The tied LM head is sharded over vocab (6288 cols/core); embedding
  weights stream chunk-by-chunk (no SBUF preload); the final world
  AllGather is split into token halves so head matmuls start early.

All matmuls run in bf16 with fp32 PSUM accumulation; the residual stream
and layernorm statistics stay fp32. LN scales and the attention scale
are folded into weight matrices on the host (exact); all bias vectors in
this problem are structurally zero (asserted).
"""

import sys

for _p in ("/opt/trn_rl_repo", "/root/.axon_site"):
    if _p not in sys.path:
        sys.path.insert(0, _p)

import numpy as np
import ml_dtypes

import concourse.bass as bass
import concourse.tile as tile
from concourse import bacc, mybir
from concourse.bass_utils import run_bass_kernel_spmd

F32 = mybir.dt.float32
BF16 = mybir.dt.bfloat16
AF = mybir.ActivationFunctionType
OP = mybir.AluOpType

B, T, D, H, L, V = 4, 1024, 1024, 16, 8, 50257
HD = D // H  # 64
EPS = 1e-5
N_CORES = 8
TOK = 512           # tokens per core
P = 128
VP = 6288           # per-core padded vocab slice (8*6288 = 50304 >= V)
NVC = (VP + 511) // 512  # vocab chunks (13; last is 144 wide)
PAIRS = [[0, 1], [2, 3], [4, 5], [6, 7]]
WORLD = [list(range(N_CORES))]
DMA_TRANSPOSE = False


def positions_for_rank(r):
    """Global positions owned by pair-rank r, in local order (increasing)."""
    tiles = [2 * j + r for j in range(4)]
    return np.concatenate([np.arange(128 * t, 128 * (t + 1)) for t in tiles])


def _ln_tm(nc, sb, stat, psT, x_ap, xn_fm, ident, eng_evac):
    """LayerNorm of token-major x_ap [128, 4, 1024] f32 -> feature-major
    bf16 xn_fm [128, 8, 512].  Scale/bias are folded into downstream
    weights on the host, so this computes plain (x - mean) * rsqrt(var)."""
    ssum = stat.tile([128, 4], F32, tag="ssum")
    ssq = stat.tile([128, 4], F32, tag="ssq")
    for t in range(4):
        nc.vector.reduce_sum(ssum[:, t : t + 1], x_ap[:, t, :], axis=mybir.AxisListType.X)
        sc = sb.tile([128, 1024], F32, tag="ln_sc")
        nc.scalar.activation(sc[:], x_ap[:, t, :], AF.Square,
                             accum_out=ssq[:, t : t + 1])
    m = stat.tile([128, 4], F32, tag="m")
    nc.vector.tensor_scalar_mul(m[:], ssum[:], 1.0 / D)
    var = stat.tile([128, 4], F32, tag="var")
    nc.vector.tensor_scalar_mul(var[:], ssq[:], 1.0 / D)
    mm = stat.tile([128, 4], F32, tag="mm")
    nc.vector.tensor_mul(mm[:], m[:], m[:])
    nc.vector.tensor_sub(var[:], var[:], mm[:])
    eps = stat.tile([128, 1], F32, tag="eps")
    nc.vector.memset(eps[:], EPS)
    std = stat.tile([128, 4], F32, tag="std")
    nc.scalar.activation(std[:], var[:], AF.Sqrt, bias=eps[:])
    rstd = stat.tile([128, 4], F32, tag="rstd")
    nc.vector.reciprocal(rstd[:], std[:])
    nmr = stat.tile([128, 4], F32, tag="nmr")
    nc.vector.tensor_mul(nmr[:], m[:], rstd[:])
    nc.vector.tensor_scalar_mul(nmr[:], nmr[:], -1.0)
    for t in range(4):
        xn = sb.tile([128, 1024], BF16, tag="ln_xn")
        nc.vector.tensor_scalar(xn[:], x_ap[:, t, :], rstd[:, t : t + 1],
                                nmr[:, t : t + 1], OP.mult, OP.add)
        if DMA_TRANSPOSE:
            for kk in range(8):
                nc.sync.dma_start_transpose(
                    xn_fm[:, kk, t * 128:(t + 1) * 128],
                    xn[:, kk * 128:(kk + 1) * 128])
        else:
            ptr = psT.tile([128, 8, 128], BF16, tag="tr")
            for kk in range(8):
                nc.tensor.transpose(ptr[:, kk, :], xn[:, kk * 128:(kk + 1) * 128], ident[:])
            eng = nc.vector if (eng_evac + t) % 2 == 0 else nc.scalar
            if eng is nc.vector:
                nc.vector.tensor_copy(xn_fm[:, :, t * 128:(t + 1) * 128], ptr[:])
            else:
                nc.scalar.copy(xn_fm[:, :, t * 128:(t + 1) * 128], ptr[:])


def build(n_layers=L, dbg=False, no_cc=False, stages=99):
    nc = bacc.Bacc("TRN2", target_bir_lowering=False, debug=False,
                   num_devices=N_CORES)

    x0_h = nc.dram_tensor("x0", [TOK, D], F32, kind="ExternalInput")
    # pre-tiled weights: [chunks, 128(p), 8(kk), 512(c)] per layer
    wqkv_h = nc.dram_tensor("wqkv", [n_layers, 6, 128, 8, 512], BF16, kind="ExternalInput")
    wp_h = nc.dram_tensor("wp", [n_layers, 2, 128, 8, 512], BF16, kind="ExternalInput")
    w1_h = nc.dram_tensor("w1", [n_layers, 8, 128, 8, 512], BF16, kind="ExternalInput")
    w2_h = nc.dram_tensor("w2", [n_layers, 2, 8, 128, 4, 512], BF16, kind="ExternalInput")
    embT_h = nc.dram_tensor("embT", [NVC, 128, 8, 512], BF16, kind="ExternalInput")
    msk_h = nc.dram_tensor("msk", [2, 128, 128], BF16, kind="ExternalInput")
    ident_h = nc.dram_tensor("identin", [128, 128], BF16, kind="ExternalInput")
    ones64_h = nc.dram_tensor("ones64", [1, 64], BF16, kind="ExternalInput")
    out_h = nc.dram_tensor("out", [N_CORES * TOK, VP], BF16, kind="ExternalOutput")

    dbg_outs = {}

    def dbg_dump(name, ap, shape, rearr=None):
        if not dbg:
            return
        t = nc.dram_tensor(f"dbg_{name}", list(shape), ap.dtype, kind="ExternalOutput")
        dst = t.ap() if rearr is None else t.ap().rearrange(rearr)
        nc.sync.dma_start(dst, ap)
        dbg_outs[name] = shape

    ag_in = [nc.dram_tensor(f"agin{l}", [128, 4096], BF16, kind="Internal")
             for l in range(n_layers)]
    ag_out = [nc.dram_tensor(f"agout{l}", [256, 4096], BF16, kind="Internal")
              for l in range(n_layers)]
    # final AG split into token halves: [128, 8(kk) x 256(tok-half)]
    agf_in = [nc.dram_tensor(f"agfin{h}", [128, 2048], BF16, kind="Internal")
              for h in range(2)]
    agf_out = [nc.dram_tensor(f"agfout{h}", [N_CORES * 128, 2048], BF16,
                              kind="Internal", addr_space="Shared")
               for h in range(2)]

    with tile.TileContext(nc) as tc:
      with tc.tile_pool(name="const", bufs=1) as constp, \
           tc.tile_pool(name="xres", bufs=1) as xresp:
        with tc.tile_pool(name="stat", bufs=2) as stat, \
             tc.tile_pool(name="sb", bufs=2) as sb, \
             tc.tile_pool(name="act", bufs=1) as actp, \
             tc.tile_pool(name="wch", bufs=5) as wch, \
             tc.tile_pool(name="pp", bufs=8) as pp, \
             tc.tile_pool(name="praw", bufs=4) as prawp, \
             tc.tile_pool(name="psA", bufs=4, space="PSUM") as psA, \
             tc.tile_pool(name="psB", bufs=3, space="PSUM") as psB, \
             tc.tile_pool(name="psT", bufs=1, space="PSUM") as psT:

            ident = constp.tile([128, 128], BF16)
            nc.sync.dma_start(ident[:], ident_h[:])
            msk = constp.tile([128, 2, 128], BF16)
            nc.sync.dma_start(msk[:], msk_h.ap().rearrange("b p q -> p b q"))
            ones64 = constp.tile([1, 64], BF16)
            nc.sync.dma_start(ones64[:], ones64_h[:])

            # residual stream, token-major fp32 [part, tok-tile, D]
            x = xresp.tile([128, 4, D], F32)
            nc.sync.dma_start(x[:], x0_h.ap().rearrange("(t p) d -> p t d", p=128))

            for l in range(n_layers):
                # ---- LN1 -> xn_fm (feature-major bf16), kick pair AllGather
                xn_fm = actp.tile([128, 8, TOK], BF16, tag="xn_fm")
                _ln_tm(nc, sb, stat, psT, x, xn_fm, ident, eng_evac=0)
                nc.sync.dma_start(
                    ag_in[l].ap().rearrange("p (kk t) -> p kk t", kk=8), xn_fm[:])
                if no_cc:
                    nc.sync.dma_start(ag_out[l][0:128, :], ag_in[l][:])
                    nc.sync.dma_start(ag_out[l][128:256, :], ag_in[l][:])
                else:
                    nc.gpsimd.collective_compute(
                        "AllGather", OP.bypass, replica_groups=PAIRS,
                        ins=[ag_in[l][:]], outs=[ag_out[l][:]])
                if l == 0:
                    dbg_dump("xn_fm0", xn_fm[:], [128, 8, TOK])

                # ---- Q^T (feature-major) from local xn_fm (covers the AG)
                if stages < 3:
                    continue
                q_fm = actp.tile([128, 8, TOK], BF16, tag="q_fm")
                for ch in range(2):
                    wt = wch.tile([128, 8, 512], BF16, tag="w", name=f"wq{l}_{ch}")
                    nc.sync.dma_start(wt[:], wqkv_h[l, ch])
                    for mi in range(4):
                        ps = psA.tile([128, TOK], F32, tag="mm")
                        for kk in range(8):
                            nc.tensor.matmul(ps[:], wt[:, kk, mi * 128:(mi + 1) * 128],
                                             xn_fm[:, kk, :], start=(kk == 0), stop=(kk == 7))
                        nc.scalar.copy(q_fm[:, ch * 4 + mi, :], ps[:])

                # ---- K/V for LOCAL tokens (bank 0) — also covers the AG
                k_all = actp.tile([128, 16, TOK], BF16, tag="k_all")
                wkt = {}
                for ch in range(2):
                    wt = wch.tile([128, 8, 512], BF16, tag="w", name=f"wk{l}_{ch}")
                    nc.sync.dma_start(wt[:], wqkv_h[l, 2 + ch])
                    wkt[ch] = wt
                    for mi in range(4):
                        ps = psA.tile([128, TOK], F32, tag="mm")
                        for kk in range(8):
                            nc.tensor.matmul(
                                ps[:], wt[:, kk, mi * 128:(mi + 1) * 128],
                                xn_fm[:, kk, :], start=(kk == 0), stop=(kk == 7))
                        nc.vector.tensor_copy(k_all[:, ch * 4 + mi, :], ps[:])

                v_all = actp.tile([128, 8, H, HD + 1], BF16, tag="v_all")
                nc.vector.memset(v_all[:, :, :, HD:HD + 1], 1.0)
                wvt = {}
                for ch in range(2):
                    wt = wch.tile([128, 8, 512], BF16, tag="w", name=f"wv{l}_{ch}")
                    nc.sync.dma_start(wt[:], wqkv_h[l, 4 + ch])
                    wvt[ch] = wt
                    for t in range(4):
                        ps = psA.tile([128, TOK], F32, tag="mm")
                        for kk in range(8):
                            nc.tensor.matmul(
                                ps[:], xn_fm[:, kk, t * 128:(t + 1) * 128],
                                wt[:, kk, :], start=(kk == 0), stop=(kk == 7))
                        nc.vector.tensor_copy(
                            v_all[:, t, ch * 8:(ch + 1) * 8, 0:HD],
                            ps[:].rearrange("p (h d) -> p h d", h=8))

                # ---- gather pair xn; recover remote half bit-exactly as
                # (bank0 + bank1) - xn_local (fp32 intermediate => exact)
                if stages < 4:
                    continue
                xn_rem = actp.tile([128, 8, TOK], BF16, tag="xn_rem")
                for kk in range(8):
                    xb0 = sb.tile([128, TOK], BF16, tag="xb0")
                    xb1 = sb.tile([128, TOK], BF16, tag="xb1")
                    nc.sync.dma_start(xb0[:], ag_out[l][0:128, kk * TOK:(kk + 1) * TOK])
                    nc.sync.dma_start(xb1[:], ag_out[l][128:256, kk * TOK:(kk + 1) * TOK])
                    tmp = sb.tile([128, TOK], F32, tag="tmp32")
                    nc.vector.tensor_add(tmp[:], xb0[:], xb1[:])
                    nc.vector.tensor_sub(xn_rem[:, kk, :], tmp[:], xn_fm[:, kk, :])

                # ---- remote K/V and attention, interleaved in groups of two
                # head pairs: the scalar-bound exp stream of one group
                # overlaps the PE-bound remote-K/V matmuls of the next
                if stages < 5:
                    continue
                o_fm = actp.tile([128, 8, TOK], BF16, tag="o_fm")

                def attn_pair(hp):
                    he, ho = 2 * hp, 2 * hp + 1
                    ave = psB.tile([P, TOK], F32, tag="acc", name=f"av{l}_{he}")
                    avo = psB.tile([P, TOK], F32, tag="acc", name=f"av{l}_{ho}")
                    for b in range(2):
                        for i in range(4):
                            n = TOK - 128 * i
                            spe = psA.tile([128, n], F32, tag="mm")
                            spo = psA.tile([128, n], F32, tag="mm")
                            nc.tensor.matmul(
                                spe[:], k_all[0:64, b * 8 + hp, i * 128:(i + 1) * 128],
                                q_fm[0:64, hp, 128 * i:TOK], start=True, stop=True)
                            nc.tensor.matmul(
                                spo[:], k_all[64:128, b * 8 + hp, i * 128:(i + 1) * 128],
                                q_fm[64:128, hp, 128 * i:TOK], start=True, stop=True)
                            # evacuate scores to SBUF f32 on DVE: frees the
                            # PSUM bank fast and lets ACT exp read from SBUF
                            pre = prawp.tile([128, n], F32, tag="praw")
                            pro = prawp.tile([128, n], F32, tag="praw")
                            nc.vector.tensor_copy(pre[:], spe[:])
                            nc.vector.tensor_copy(pro[:], spo[:])
                            pte = pp.tile([128, n], BF16, tag="p")
                            pto = pp.tile([128, n], BF16, tag="p")
                            nc.scalar.activation(pte[:], pre[:], AF.Exp)
                            nc.scalar.activation(pto[:], pro[:], AF.Exp)
                            # causal mask on the diagonal 128-col block
                            nc.vector.tensor_mul(pte[:, 0:128], pte[:, 0:128], msk[:, b, :])
                            nc.vector.tensor_mul(pto[:, 0:128], pto[:, 0:128], msk[:, b, :])
                            nc.tensor.matmul(
                                ave[0:HD + 1, 128 * i:TOK],
                                v_all[:, b * 4 + i, he, :], pte[:],
                                start=(b == 0 and i == 0), stop=(b == 1 and i == 3))
                            nc.tensor.matmul(
                                avo[0:HD + 1, 128 * i:TOK],
                                v_all[:, b * 4 + i, ho, :], pto[:],
                                start=(b == 0 and i == 0), stop=(b == 1 and i == 3))
                    # denominators -> broadcast (ones64 MMs) -> fast approx
                    # reciprocal (~18 bits, plenty vs bf16 downstream)
                    den = sb.tile([1, 2, TOK], BF16, tag="den")
                    nc.vector.tensor_copy(den[0:1, 0, :], ave[HD:HD + 1, :])
                    nc.vector.tensor_copy(den[0:1, 1, :], avo[HD:HD + 1, :])
                    bp = psA.tile([128, TOK], F32, tag="mm")
                    nc.tensor.matmul(bp[0:64, :], ones64[:], den[0:1, 0, :],
                                     start=True, stop=True)
                    nc.tensor.matmul(bp[64:128, :], ones64[:], den[0:1, 1, :],
                                     start=True, stop=True)
                    rb = sb.tile([128, TOK], F32, tag="rb")
                    nc.vector.reciprocal_approx_fast(rb[:], bp[:])
                    nc.vector.tensor_tensor(o_fm[0:64, hp, :], ave[0:HD, :],
                                            rb[0:64, :], OP.mult)
                    nc.vector.tensor_tensor(o_fm[64:128, hp, :], avo[0:HD, :],
                                            rb[64:128, :], OP.mult)

                # remote K/V first (dense PE stream), attention after: the PE
                # queue is strict FIFO, so attention's exp-gated AV matmuls
                # must not sit in front of independent dense work
                for ch in range(2):
                    for mi in range(4):
                        ps = psA.tile([128, TOK], F32, tag="mm")
                        for kk in range(8):
                            nc.tensor.matmul(
                                ps[:], wkt[ch][:, kk, mi * 128:(mi + 1) * 128],
                                xn_rem[:, kk, :], start=(kk == 0), stop=(kk == 7))
                        nc.vector.tensor_copy(k_all[:, 8 + ch * 4 + mi, :], ps[:])
                for ch in range(2):
                    for t in range(4):
                        ps = psA.tile([128, TOK], F32, tag="mm")
                        for kk in range(8):
                            nc.tensor.matmul(
                                ps[:], xn_rem[:, kk, t * 128:(t + 1) * 128],
                                wvt[ch][:, kk, :], start=(kk == 0), stop=(kk == 7))
                        nc.vector.tensor_copy(
                            v_all[:, 4 + t, ch * 8:(ch + 1) * 8, 0:HD],
                            ps[:].rearrange("p (h d) -> p h d", h=8))
                for hp in range(H // 2):
                    attn_pair(hp)
                if l == 0:
                    dbg_dump("k_all0", k_all[:], [128, 16, TOK])
                    dbg_dump("v_all0", v_all[:], [128, 8, H, HD + 1])
                    dbg_dump("o_fm0", o_fm[:], [128, 8, TOK])

                # ---- projection (token-major) + residual
                if stages < 7:
                    continue
                for ch in range(2):
                    wt = wch.tile([128, 8, 512], BF16, tag="w", name=f"wpj{l}_{ch}")
                    nc.sync.dma_start(wt[:], wp_h[l, ch])
                    for t in range(4):
                        ps = psA.tile([128, 512], F32, tag="mm")
                        for kk in range(8):
                            nc.tensor.matmul(
                                ps[:], o_fm[:, kk, t * 128:(t + 1) * 128],
                                wt[:, kk, :], start=(kk == 0), stop=(kk == 7))
                        nc.vector.tensor_add(x[:, t, ch * 512:(ch + 1) * 512],
                                             x[:, t, ch * 512:(ch + 1) * 512], ps[:])
                if l == 0:
                    dbg_dump("xattn0", x[:], [128, 4, D])

                # ---- LN2 -> xn2_fm
                if stages < 8:
                    continue
                xn2_fm = actp.tile([128, 8, TOK], BF16, tag="xn2_fm")
                _ln_tm(nc, sb, stat, psT, x, xn2_fm, ident, eng_evac=1)

                # ---- FFN: ff1 full-token, ff2 in token halves
                h_sb = actp.tile([128, 32, TOK], BF16, tag="h_sb")
                for mc in range(8):
                    wt = wch.tile([128, 8, 512], BF16, tag="w", name=f"w1_{l}_{mc}")
                    nc.sync.dma_start(wt[:], w1_h[l, mc])
                    for mi in range(4):
                        ps = psA.tile([128, TOK], F32, tag="mm")
                        for kk in range(8):
                            nc.tensor.matmul(
                                ps[:], wt[:, kk, mi * 128:(mi + 1) * 128],
                                xn2_fm[:, kk, :], start=(kk == 0), stop=(kk == 7))
                        nc.scalar.activation(h_sb[:, mc * 4 + mi, :], ps[:], AF.Gelu)
                for half in range(2):
                    for nch in range(2):
                        acc = [psB.tile([128, 512], F32, tag="acc",
                                        name=f"acc{l}_{half}_{nch}_{a}") for a in range(2)]
                        for kkc in range(8):
                            w2t = wch.tile([128, 4, 512], BF16, tag="w",
                                           name=f"w2_{l}_{half}_{nch}_{kkc}")
                            nc.sync.dma_start(w2t[:], w2_h[l, nch, kkc])
                            for kki in range(4):
                                for mi in range(2):
                                    nc.tensor.matmul(
                                        acc[mi][:],
                                        h_sb[:, kkc * 4 + kki,
                                             half * 256 + mi * 128:half * 256 + (mi + 1) * 128],
                                        w2t[:, kki, :],
                                        start=(kkc == 0 and kki == 0),
                                        stop=(kkc == 7 and kki == 3))
                        for mi in range(2):
                            t = half * 2 + mi
                            nc.vector.tensor_add(x[:, t, nch * 512:(nch + 1) * 512],
                                                 x[:, t, nch * 512:(nch + 1) * 512],
                                                 acc[mi][:])
                if l == 0:
                    dbg_dump("xlayer0", x[:], [128, 4, D])

            if stages < 9:
                # early-exit build for bisection: dump residual so work isn't DCE'd
                xdump = nc.dram_tensor("xdump", [128, 4, D], F32, kind="ExternalOutput")
                nc.sync.dma_start(xdump.ap(), x[:])

        # ---- final LN + LM head phase (separate pools; trunk SBUF released)
        with tc.tile_pool(name="stat2", bufs=2) as stat2, \
             tc.tile_pool(name="sb2", bufs=2) as sb2, \
             tc.tile_pool(name="hd", bufs=1) as hd, \
             tc.tile_pool(name="emb", bufs=3) as epool, \
             tc.tile_pool(name="hout", bufs=4) as hout, \
             tc.tile_pool(name="psT2", bufs=1, space="PSUM") as psT2, \
             tc.tile_pool(name="psH", bufs=4, space="PSUM") as psH:
            if stages >= 9:
                xnf_fm = hd.tile([128, 8, TOK], BF16)
                _ln_tm(nc, sb2, stat2, psT2, x, xnf_fm, ident, eng_evac=0)
                # world AllGather split into token halves so head MMs for the
                # first half start while the second half is still in flight
                for hh in range(2):
                    nc.sync.dma_start(
                        agf_in[hh].ap().rearrange("p (kk t) -> p kk t", kk=8),
                        xnf_fm[:, :, hh * 256:(hh + 1) * 256])
                    if no_cc:
                        for r_ in range(N_CORES):
                            nc.sync.dma_start(
                                agf_out[hh][r_ * 128:(r_ + 1) * 128, :], agf_in[hh][:])
                    else:
                        nc.gpsimd.collective_compute(
                            "AllGather", OP.bypass, replica_groups=WORLD,
                            ins=[agf_in[hh][:]], outs=[agf_out[hh][:]])
                if dbg:
                    dbg_dump("xnf_fm", xnf_fm[:], [128, 8, TOK])
                xn_all = hd.tile([128, 64, TOK], BF16)
                for hh in range(2):
                    for r_ in range(8):
                        nc.sync.dma_start(
                            xn_all[:, r_ * 8:(r_ + 1) * 8, hh * 256:(hh + 1) * 256],
                            agf_out[hh][r_ * 128:(r_ + 1) * 128, :].rearrange(
                                "p (kk t) -> p kk t", kk=8))
                nchunks = [(i * 512, min(512, VP - i * 512)) for i in range(NVC)]
                for ni, (n0, nsz) in enumerate(nchunks):
                    et = epool.tile([128, 8, 512], BF16, tag="emb")
                    nc.sync.dma_start(et[:], embT_h[ni])
                    # token half 0 (tiles 0,1 of every rank) first: available
                    # as soon as the first half-AG lands
                    for mi in ([m for m in range(32) if m % 4 < 2]
                               + [m for m in range(32) if m % 4 >= 2]):
                        r, t = mi // 4, mi % 4
                        ps = psH.tile([128, nsz], F32, tag="h")
                        for kk in range(8):
                            nc.tensor.matmul(
                                ps[:], xn_all[:, r * 8 + kk, t * 128:(t + 1) * 128],
                                et[:, kk, 0:nsz],
                                start=(kk == 0), stop=(kk == 7))
                        osb = hout.tile([128, nsz], BF16, tag="o")
                        if mi % 2 == 0:
                            nc.vector.tensor_copy(osb[:], ps[:])
                        else:
                            nc.scalar.copy(osb[:], ps[:])
                        nc.sync.dma_start(out_h[mi * 128:(mi + 1) * 128, n0:n0 + nsz], osb[:])

    nc.compile()
    return nc, dbg_outs


def _fm_tile_w(w):
    """[1024, nch*512] -> [nch, 128, 8, 512]; tile[j,p,kk,c] = w[kk*128+p, j*512+c]."""
    din, dout = w.shape
    nch = dout // 512
    r = w.reshape(8, 128, nch, 512)
    return np.ascontiguousarray(r.transpose(2, 1, 0, 3))


def prepare_inputs(idx, tok_emb, pos_emb, qkv_w, qkv_b, proj_w, proj_b,
                   ff1_w, ff1_b, ff2_w, ff2_b, ln1_s, ln1_b, ln2_s, ln2_b,
                   lnf_s, lnf_b, n_layers=L):
    """Host-side sharding/folding. Returns per-core in_maps."""
    bf = ml_dtypes.bfloat16
    for name, v in (("qkv_b", qkv_b), ("proj_b", proj_b), ("ff1_b", ff1_b),
                    ("ff2_b", ff2_b), ("ln1_b", ln1_b), ("ln2_b", ln2_b),
                    ("lnf_b", lnf_b)):
        assert np.allclose(np.asarray(v), 0.0), f"nonzero {name} not supported"

    idx = np.asarray(idx)
    tok_emb = np.asarray(tok_emb, np.float32)
    pos_emb = np.asarray(pos_emb, np.float32)
    scale = 1.0 / np.sqrt(HD)

    # fold LN scales + attention scale into weights (exact)
    wqkv = (np.asarray(qkv_w[:n_layers], np.float32)
            * np.asarray(ln1_s[:n_layers], np.float32)[:, :, None]).copy()
    wqkv[:, :, :D] *= scale
    w1 = (np.asarray(ff1_w[:n_layers], np.float32)
          * np.asarray(ln2_s[:n_layers], np.float32)[:, :, None])
    wp = np.asarray(proj_w[:n_layers], np.float32)
    w2 = np.asarray(ff2_w[:n_layers], np.float32)
    embT_full = (tok_emb * np.asarray(lnf_s, np.float32)[None, :]).T  # [D, V]
    embT_pad = np.zeros((D, N_CORES * VP), np.float32)
    embT_pad[:, :V] = embT_full

    # pre-tiled weight arrays (contiguous 1MB DMA bursts on device)
    wqkv_t = np.stack([_fm_tile_w(wqkv[l]) for l in range(n_layers)]).astype(bf)
    wp_t = np.stack([_fm_tile_w(wp[l]) for l in range(n_layers)]).astype(bf)
    w1_t = np.stack([_fm_tile_w(w1[l]) for l in range(n_layers)]).astype(bf)
    # w2: [4096, 1024] -> [2(nch), 8(kkc), 128(p), 4(kki), 512(c)]
    w2_t = np.stack([
        np.ascontiguousarray(
            w2[l].reshape(8, 4, 128, 2, 512).transpose(3, 0, 2, 1, 4))
        for l in range(n_layers)]).astype(bf)

    ident = np.eye(128, dtype=bf)
    ones64 = np.ones((1, 64), bf)

    tri = np.tril(np.ones((128, 128), np.float32)).T  # [kt, q] valid kt<=q
    # core-relative banks: slot0 = local diagonal (triangular for both
    # ranks); slot1 = remote diagonal (all-masked for r=0, visible for r=1)
    msk_r = [np.zeros((2, 128, 128), np.float32) for _ in range(2)]
    msk_r[0][0] = tri
    msk_r[0][1] = 0.0
    msk_r[1][0] = tri
    msk_r[1][1] = 1.0

    in_maps = []
    for c in range(N_CORES):
        b, r = c // 2, c % 2
        pos = positions_for_rank(r)
        x0 = tok_emb[idx[b, pos]] + pos_emb[pos]
        # per-core vocab slice, padded to 13*512 cols for uniform DMA
        esl = np.zeros((D, NVC * 512), np.float32)
        esl[:, :VP] = embT_pad[:, c * VP:(c + 1) * VP]
        embT_tiles = np.ascontiguousarray(
            esl.reshape(8, 128, NVC, 512).transpose(2, 1, 0, 3)).astype(bf)
        in_maps.append({
            "x0": np.ascontiguousarray(x0, np.float32),
            "wqkv": wqkv_t, "wp": wp_t, "w1": w1_t, "w2": w2_t,
            "embT": embT_tiles,
            "msk": msk_r[r].astype(bf),
            "identin": ident,
            "ones64": ones64,
        })
    return in_maps


def assemble_output(results):
    """Per-core [4096, VP] bf16 -> full logits [B, T, V] f32."""
    logits = np.empty((B, T, V), np.float32)
    pos_r = [positions_for_rank(0), positions_for_rank(1)]
    for c in range(N_CORES):
        out_c = np.asarray(results[c]["out"], np.float32)  # [4096, VP]
        v0 = c * VP
        ncols = min(VP, V - v0)
        if ncols <= 0:
            continue
        for r in range(N_CORES):
            bb, rr = r // 2, r % 2
            logits[bb, pos_r[rr], v0:v0 + ncols] = \
                out_c[r * TOK:(r + 1) * TOK, :ncols]
    return logits


_NC_CACHE = {}


def _get_nc(n_layers=L, dbg=False):
    key = (n_layers, dbg)
    if key not in _NC_CACHE:
        _NC_CACHE[key] = build(n_layers=n_layers, dbg=dbg)
    return _NC_CACHE[key]


def kernel(**inputs):
    in_maps = prepare_inputs(**inputs)
    nc, _ = _get_nc()
    res = run_bass_kernel_spmd(nc, in_maps, core_ids=list(range(N_CORES)))
    return assemble_output(res.results)


# revision 19
# speedup vs baseline: 1.0043x; 1.0043x over previous
"""Trainium2 Bass kernel for nn_AtomsGPT (GPT-2-style dense transformer).

B=4, T=1024, D=1024, H=16 heads, L=8 layers, V=50257, tied LM head.

Sharding (8 NeuronCores):
- Token-data-parallel trunk: core c owns batch c//2, pair-rank r=c%2.
  Rank r takes the even (r=0) / odd (r=1) 128-position tiles of the
  sequence, interleaved for causal-attention load balance.
- Per layer the pair exchanges LN1 outputs via a 2-rank AllGather.  The
  local K/V matmuls run from the local xn while the AllGather is in
  flight; the remote half is recovered bit-exactly as
  (bank0+bank1) - xn_local in fp32, keeping the program rank-agnostic
  (k_all/v_all banks are core-relative: bank0=local, bank1=remote).
- Attention processes head PAIRS: the two 64-wide score matmuls use
  disjoint PE row groups (tile_position auto-derived) and run
  concurrently.  Causal masks are per-core input data (local diagonal =
  triangular for both ranks; remote diagonal = all-ones / all-zeros).
- The tied LM head is sharded over vocab (6288 cols/core); embedding
  weights stream chunk-by-chunk (no SBUF preload); the final world
  AllGather is split into token halves so head matmuls start early.

All matmuls run in bf16 with fp32 PSUM accumulation; the residual stream
and layernorm statistics stay fp32. LN scales and the attention scale
are folded into weight matrices on the host (exact); all bias vectors in
this problem are structurally zero (asserted).
"""

import sys

for _p in ("/opt/trn_rl_repo", "/root/.axon_site"):
    if _p not in sys.path:
        sys.path.insert(0, _p)

import numpy as np
import ml_dtypes

import concourse.bass as bass
import concourse.tile as tile
from concourse import bacc, mybir
from concourse.bass_utils import run_bass_kernel_spmd

F32 = mybir.dt.float32
BF16 = mybir.dt.bfloat16
AF = mybir.ActivationFunctionType
OP = mybir.AluOpType

B, T, D, H, L, V = 4, 1024, 1024, 16, 8, 50257
HD = D // H  # 64
EPS = 1e-5
N_CORES = 8
TOK = 512           # tokens per core
P = 128
VP = 6288           # per-core padded vocab slice (8*6288 = 50304 >= V)
NVC = (VP + 511) // 512  # vocab chunks (13; last is 144 wide)
PAIRS = [[0, 1], [2, 3], [4, 5], [6, 7]]
WORLD = [list(range(N_CORES))]
DMA_TRANSPOSE = False


def positions_for_rank(r):
    """Global positions owned by pair-rank r, in local order (increasing)."""
    tiles = [2 * j + r for j in range(4)]
    return np.concatenate([np.arange(128 * t, 128 * (t + 1)) for t in tiles])


def _ln_tm(nc, sb, stat, psT, x_ap, xn_fm, ident, eng_evac):
    """LayerNorm of token-major x_ap [128, 4, 1024] f32 -> feature-major
    bf16 xn_fm [128, 8, 512].  Scale/bias are folded into downstream
    weights on the host, so this computes plain (x - mean) * rsqrt(var)."""
    ssum = stat.tile([128, 4], F32, tag="ssum")
    ssq = stat.tile([128, 4], F32, tag="ssq")
    for t in range(4):
        nc.vector.reduce_sum(ssum[:, t : t + 1], x_ap[:, t, :], axis=mybir.AxisListType.X)
        sc = sb.tile([128, 1024], F32, tag="ln_sc")
        nc.scalar.activation(sc[:], x_ap[:, t, :], AF.Square,
                             accum_out=ssq[:, t : t + 1])
    m = stat.tile([128, 4], F32, tag="m")
    nc.vector.tensor_scalar_mul(m[:], ssum[:], 1.0 / D)
    var = stat.tile([128, 4], F32, tag="var")
    nc.vector.tensor_scalar_mul(var[:], ssq[:], 1.0 / D)
    mm = stat.tile([128, 4], F32, tag="mm")
    nc.vector.tensor_mul(mm[:], m[:], m[:])
    nc.vector.tensor_sub(var[:], var[:], mm[:])
    eps = stat.tile([128, 1], F32, tag="eps")
    nc.vector.memset(eps[:], EPS)
    std = stat.tile([128, 4], F32, tag="std")
    nc.scalar.activation(std[:], var[:], AF.Sqrt, bias=eps[:])
    rstd = stat.tile([128, 4], F32, tag="rstd")
    nc.vector.reciprocal(rstd[:], std[:])
    nmr = stat.tile([128, 4], F32, tag="nmr")
    nc.vector.tensor_mul(nmr[:], m[:], rstd[:])
    nc.vector.tensor_scalar_mul(nmr[:], nmr[:], -1.0)
    for t in range(4):
        xn = sb.tile([128, 1024], BF16, tag="ln_xn")
        nc.vector.tensor_scalar(xn[:], x_ap[:, t, :], rstd[:, t : t + 1],
                                nmr[:, t : t + 1], OP.mult, OP.add)
        if DMA_TRANSPOSE:
            for kk in range(8):
                nc.sync.dma_start_transpose(
                    xn_fm[:, kk, t * 128:(t + 1) * 128],
                    xn[:, kk * 128:(kk + 1) * 128])
        else:
            ptr = psT.tile([128, 8, 128], BF16, tag="tr")
            for kk in range(8):
                nc.tensor.transpose(ptr[:, kk, :], xn[:, kk * 128:(kk + 1) * 128], ident[:])
            eng = nc.vector if (eng_evac + t) % 2 == 0 else nc.scalar
            if eng is nc.vector:
                nc.vector.tensor_copy(xn_fm[:, :, t * 128:(t + 1) * 128], ptr[:])
            else:
                nc.scalar.copy(xn_fm[:, :, t * 128:(t + 1) * 128], ptr[:])


def build(n_layers=L, dbg=False, no_cc=False, stages=99):
    nc = bacc.Bacc("TRN2", target_bir_lowering=False, debug=False,
                   num_devices=N_CORES)

    x0_h = nc.dram_tensor("x0", [TOK, D], F32, kind="ExternalInput")
    # pre-tiled weights: [chunks, 128(p), 8(kk), 512(c)] per layer
    wqkv_h = nc.dram_tensor("wqkv", [n_layers, 6, 128, 8, 512], BF16, kind="ExternalInput")
    wp_h = nc.dram_tensor("wp", [n_layers, 2, 128, 8, 512], BF16, kind="ExternalInput")
    w1_h = nc.dram_tensor("w1", [n_layers, 8, 128, 8, 512], BF16, kind="ExternalInput")
    w2_h = nc.dram_tensor("w2", [n_layers, 2, 8, 128, 4, 512], BF16, kind="ExternalInput")
    embT_h = nc.dram_tensor("embT", [NVC, 128, 8, 512], BF16, kind="ExternalInput")
    msk_h = nc.dram_tensor("msk", [2, 128, 128], BF16, kind="ExternalInput")
    ident_h = nc.dram_tensor("identin", [128, 128], BF16, kind="ExternalInput")
    ones64_h = nc.dram_tensor("ones64", [1, 64], BF16, kind="ExternalInput")
    out_h = nc.dram_tensor("out", [N_CORES * TOK, VP], BF16, kind="ExternalOutput")

    dbg_outs = {}

    def dbg_dump(name, ap, shape, rearr=None):
        if not dbg:
            return
        t = nc.dram_tensor(f"dbg_{name}", list(shape), ap.dtype, kind="ExternalOutput")
        dst = t.ap() if rearr is None else t.ap().rearrange(rearr)
        nc.sync.dma_start(dst, ap)
        dbg_outs[name] = shape

    ag_in = [nc.dram_tensor(f"agin{l}", [128, 4096], BF16, kind="Internal")
             for l in range(n_layers)]
    ag_out = [nc.dram_tensor(f"agout{l}", [256, 4096], BF16, kind="Internal")
              for l in range(n_layers)]
    # final AG split into token halves: [128, 8(kk) x 256(tok-half)]
    agf_in = [nc.dram_tensor(f"agfin{h}", [128, 2048], BF16, kind="Internal")
              for h in range(2)]
    agf_out = [nc.dram_tensor(f"agfout{h}", [N_CORES * 128, 2048], BF16,
                              kind="Internal", addr_space="Shared")
               for h in range(2)]

    with tile.TileContext(nc) as tc:
      with tc.tile_pool(name="const", bufs=1) as constp, \
           tc.tile_pool(name="xres", bufs=1) as xresp:
        with tc.tile_pool(name="stat", bufs=2) as stat, \
             tc.tile_pool(name="sb", bufs=2) as sb, \
             tc.tile_pool(name="act", bufs=1) as actp, \
             tc.tile_pool(name="wch", bufs=6) as wch, \
             tc.tile_pool(name="pp", bufs=8) as pp, \
             tc.tile_pool(name="psA", bufs=4, space="PSUM") as psA, \
             tc.tile_pool(name="psB", bufs=3, space="PSUM") as psB, \
             tc.tile_pool(name="psT", bufs=1, space="PSUM") as psT:

            ident = constp.tile([128, 128], BF16)
            nc.sync.dma_start(ident[:], ident_h[:])
            msk = constp.tile([128, 2, 128], BF16)
            nc.sync.dma_start(msk[:], msk_h.ap().rearrange("b p q -> p b q"))
            ones64 = constp.tile([1, 64], BF16)
            nc.sync.dma_start(ones64[:], ones64_h[:])

            # residual stream, token-major fp32 [part, tok-tile, D]
            x = xresp.tile([128, 4, D], F32)
            nc.sync.dma_start(x[:], x0_h.ap().rearrange("(t p) d -> p t d", p=128))

            for l in range(n_layers):
                # ---- LN1 -> xn_fm (feature-major bf16), kick pair AllGather
                xn_fm = actp.tile([128, 8, TOK], BF16, tag="xn_fm")
                _ln_tm(nc, sb, stat, psT, x, xn_fm, ident, eng_evac=0)
                nc.sync.dma_start(
                    ag_in[l].ap().rearrange("p (kk t) -> p kk t", kk=8), xn_fm[:])
                if no_cc:
                    nc.sync.dma_start(ag_out[l][0:128, :], ag_in[l][:])
                    nc.sync.dma_start(ag_out[l][128:256, :], ag_in[l][:])
                else:
                    nc.gpsimd.collective_compute(
                        "AllGather", OP.bypass, replica_groups=PAIRS,
                        ins=[ag_in[l][:]], outs=[ag_out[l][:]])
                if l == 0:
                    dbg_dump("xn_fm0", xn_fm[:], [128, 8, TOK])

                # ---- Q^T (feature-major) from local xn_fm (covers the AG)
                if stages < 3:
                    continue
                q_fm = actp.tile([128, 8, TOK], BF16, tag="q_fm")
                for ch in range(2):
                    wt = wch.tile([128, 8, 512], BF16, tag="w", name=f"wq{l}_{ch}")
                    nc.sync.dma_start(wt[:], wqkv_h[l, ch])
                    for mi in range(4):
                        ps = psA.tile([128, TOK], F32, tag="mm")
                        for kk in range(8):
                            nc.tensor.matmul(ps[:], wt[:, kk, mi * 128:(mi + 1) * 128],
                                             xn_fm[:, kk, :], start=(kk == 0), stop=(kk == 7))
                        nc.scalar.copy(q_fm[:, ch * 4 + mi, :], ps[:])

                # ---- K/V for LOCAL tokens (bank 0) — also covers the AG
                k_all = actp.tile([128, 16, TOK], BF16, tag="k_all")
                wkt = {}
                for ch in range(2):
                    wt = wch.tile([128, 8, 512], BF16, tag="w", name=f"wk{l}_{ch}")
                    nc.sync.dma_start(wt[:], wqkv_h[l, 2 + ch])
                    wkt[ch] = wt
                    for mi in range(4):
                        ps = psA.tile([128, TOK], F32, tag="mm")
                        for kk in range(8):
                            nc.tensor.matmul(
                                ps[:], wt[:, kk, mi * 128:(mi + 1) * 128],
                                xn_fm[:, kk, :], start=(kk == 0), stop=(kk == 7))
                        nc.vector.tensor_copy(k_all[:, ch * 4 + mi, :], ps[:])

                v_all = actp.tile([128, 8, H, HD + 1], BF16, tag="v_all")
                nc.vector.memset(v_all[:, :, :, HD:HD + 1], 1.0)
                wvt = {}
                for ch in range(2):
                    wt = wch.tile([128, 8, 512], BF16, tag="w", name=f"wv{l}_{ch}")
                    nc.sync.dma_start(wt[:], wqkv_h[l, 4 + ch])
                    wvt[ch] = wt
                    for t in range(4):
                        ps = psA.tile([128, TOK], F32, tag="mm")
                        for kk in range(8):
                            nc.tensor.matmul(
                                ps[:], xn_fm[:, kk, t * 128:(t + 1) * 128],
                                wt[:, kk, :], start=(kk == 0), stop=(kk == 7))
                        nc.vector.tensor_copy(
                            v_all[:, t, ch * 8:(ch + 1) * 8, 0:HD],
                            ps[:].rearrange("p (h d) -> p h d", h=8))

                # ---- gather pair xn; recover remote half bit-exactly as
                # (bank0 + bank1) - xn_local (fp32 intermediate => exact)
                if stages < 4:
                    continue
                xn_rem = actp.tile([128, 8, TOK], BF16, tag="xn_rem")
                for kk in range(8):
                    xb0 = sb.tile([128, TOK], BF16, tag="xb0")
                    xb1 = sb.tile([128, TOK], BF16, tag="xb1")
                    nc.sync.dma_start(xb0[:], ag_out[l][0:128, kk * TOK:(kk + 1) * TOK])
                    nc.sync.dma_start(xb1[:], ag_out[l][128:256, kk * TOK:(kk + 1) * TOK])
                    tmp = sb.tile([128, TOK], F32, tag="tmp32")
                    nc.vector.tensor_add(tmp[:], xb0[:], xb1[:])
                    nc.vector.tensor_sub(xn_rem[:, kk, :], tmp[:], xn_fm[:, kk, :])

                # ---- remote K/V and attention, interleaved in groups of two
                # head pairs: the scalar-bound exp stream of one group
                # overlaps the PE-bound remote-K/V matmuls of the next
                if stages < 5:
                    continue
                o_fm = actp.tile([128, 8, TOK], BF16, tag="o_fm")

                def attn_pair(hp):
                    he, ho = 2 * hp, 2 * hp + 1
                    ave = psB.tile([P, TOK], F32, tag="acc", name=f"av{l}_{he}")
                    avo = psB.tile([P, TOK], F32, tag="acc", name=f"av{l}_{ho}")
                    for b in range(2):
                        for i in range(4):
                            n = TOK - 128 * i
                            spe = psA.tile([128, n], F32, tag="mm")
                            spo = psA.tile([128, n], F32, tag="mm")
                            nc.tensor.matmul(
                                spe[:], k_all[0:64, b * 8 + hp, i * 128:(i + 1) * 128],
                                q_fm[0:64, hp, 128 * i:TOK], start=True, stop=True)
                            nc.tensor.matmul(
                                spo[:], k_all[64:128, b * 8 + hp, i * 128:(i + 1) * 128],
                                q_fm[64:128, hp, 128 * i:TOK], start=True, stop=True)
                            pte = pp.tile([128, n], BF16, tag="p")
                            pto = pp.tile([128, n], BF16, tag="p")
                            nc.scalar.activation(pte[:], spe[:], AF.Exp)
                            nc.scalar.activation(pto[:], spo[:], AF.Exp)
                            # causal mask on the diagonal 128-col block
                            nc.vector.tensor_mul(pte[:, 0:128], pte[:, 0:128], msk[:, b, :])
                            nc.vector.tensor_mul(pto[:, 0:128], pto[:, 0:128], msk[:, b, :])
                            nc.tensor.matmul(
                                ave[0:HD + 1, 128 * i:TOK],
                                v_all[:, b * 4 + i, he, :], pte[:],
                                start=(b == 0 and i == 0), stop=(b == 1 and i == 3))
                            nc.tensor.matmul(
                                avo[0:HD + 1, 128 * i:TOK],
                                v_all[:, b * 4 + i, ho, :], pto[:],
                                start=(b == 0 and i == 0), stop=(b == 1 and i == 3))
                    # denominators -> broadcast (ones64 MMs) -> fast approx
                    # reciprocal (~18 bits, plenty vs bf16 downstream)
                    den = sb.tile([1, 2, TOK], BF16, tag="den")
                    nc.vector.tensor_copy(den[0:1, 0, :], ave[HD:HD + 1, :])
                    nc.vector.tensor_copy(den[0:1, 1, :], avo[HD:HD + 1, :])
                    bp = psA.tile([128, TOK], F32, tag="mm")
                    nc.tensor.matmul(bp[0:64, :], ones64[:], den[0:1, 0, :],
                                     start=True, stop=True)
                    nc.tensor.matmul(bp[64:128, :], ones64[:], den[0:1, 1, :],
                                     start=True, stop=True)
                    rb = sb.tile([128, TOK], F32, tag="rb")
                    nc.vector.reciprocal_approx_fast(rb[:], bp[:])
                    nc.vector.tensor_tensor(o_fm[0:64, hp, :], ave[0:HD, :],
                                            rb[0:64, :], OP.mult)
                    nc.vector.tensor_tensor(o_fm[64:128, hp, :], avo[0:HD, :],
                                            rb[64:128, :], OP.mult)

                # remote K/V first (dense PE stream), attention after: the PE
                # queue is strict FIFO, so attention's exp-gated AV matmuls
                # must not sit in front of independent dense work
                for ch in range(2):
                    for mi in range(4):
                        ps = psA.tile([128, TOK], F32, tag="mm")
                        for kk in range(8):
                            nc.tensor.matmul(
                                ps[:], wkt[ch][:, kk, mi * 128:(mi + 1) * 128],
                                xn_rem[:, kk, :], start=(kk == 0), stop=(kk == 7))
                        nc.vector.tensor_copy(k_all[:, 8 + ch * 4 + mi, :], ps[:])
                for ch in range(2):
                    for t in range(4):
                        ps = psA.tile([128, TOK], F32, tag="mm")
                        for kk in range(8):
                            nc.tensor.matmul(
                                ps[:], xn_rem[:, kk, t * 128:(t + 1) * 128],
                                wvt[ch][:, kk, :], start=(kk == 0), stop=(kk == 7))
                        nc.vector.tensor_copy(
                            v_all[:, 4 + t, ch * 8:(ch + 1) * 8, 0:HD],
                            ps[:].rearrange("p (h d) -> p h d", h=8))
                for hp in range(H // 2):
                    attn_pair(hp)
                if l == 0:
                    dbg_dump("k_all0", k_all[:], [128, 16, TOK])
                    dbg_dump("v_all0", v_all[:], [128, 8, H, HD + 1])
                    dbg_dump("o_fm0", o_fm[:], [128, 8, TOK])

                # ---- projection (token-major) + residual
                if stages < 7:
                    continue
                for ch in range(2):
                    wt = wch.tile([128, 8, 512], BF16, tag="w", name=f"wpj{l}_{ch}")
                    nc.sync.dma_start(wt[:], wp_h[l, ch])
                    for t in range(4):
                        ps = psA.tile([128, 512], F32, tag="mm")
                        for kk in range(8):
                            nc.tensor.matmul(
                                ps[:], o_fm[:, kk, t * 128:(t + 1) * 128],
                                wt[:, kk, :], start=(kk == 0), stop=(kk == 7))
                        nc.vector.tensor_add(x[:, t, ch * 512:(ch + 1) * 512],
                                             x[:, t, ch * 512:(ch + 1) * 512], ps[:])
                if l == 0:
                    dbg_dump("xattn0", x[:], [128, 4, D])

                # ---- LN2 -> xn2_fm
                if stages < 8:
                    continue
                xn2_fm = actp.tile([128, 8, TOK], BF16, tag="xn2_fm")
                _ln_tm(nc, sb, stat, psT, x, xn2_fm, ident, eng_evac=1)

                # ---- FFN: ff1 full-token, ff2 in token halves
                h_sb = actp.tile([128, 32, TOK], BF16, tag="h_sb")
                for mc in range(8):
                    wt = wch.tile([128, 8, 512], BF16, tag="w", name=f"w1_{l}_{mc}")
                    nc.sync.dma_start(wt[:], w1_h[l, mc])
                    for mi in range(4):
                        ps = psA.tile([128, TOK], F32, tag="mm")
                        for kk in range(8):
                            nc.tensor.matmul(
                                ps[:], wt[:, kk, mi * 128:(mi + 1) * 128],
                                xn2_fm[:, kk, :], start=(kk == 0), stop=(kk == 7))
                        nc.scalar.activation(h_sb[:, mc * 4 + mi, :], ps[:], AF.Gelu)
                for half in range(2):
                    for nch in range(2):
                        acc = [psB.tile([128, 512], F32, tag="acc",
                                        name=f"acc{l}_{half}_{nch}_{a}") for a in range(2)]
                        for kkc in range(8):
                            w2t = wch.tile([128, 4, 512], BF16, tag="w",
                                           name=f"w2_{l}_{half}_{nch}_{kkc}")
                            nc.sync.dma_start(w2t[:], w2_h[l, nch, kkc])
                            for kki in range(4):
                                for mi in range(2):
                                    nc.tensor.matmul(
                                        acc[mi][:],
                                        h_sb[:, kkc * 4 + kki,
                                             half * 256 + mi * 128:half * 256 + (mi + 1) * 128],
                                        w2t[:, kki, :],
                                        start=(kkc == 0 and kki == 0),
                                        stop=(kkc == 7 and kki == 3))
                        for mi in range(2):
                            t = half * 2 + mi
                            nc.vector.tensor_add(x[:, t, nch * 512:(nch + 1) * 512],
                                                 x[:, t, nch * 512:(nch + 1) * 512],
                                                 acc[mi][:])
                if l == 0:
                    dbg_dump("xlayer0", x[:], [128, 4, D])

            if stages < 9:
                # early-exit build for bisection: dump residual so work isn't DCE'd
                xdump = nc.dram_tensor("xdump", [128, 4, D], F32, kind="ExternalOutput")
                nc.sync.dma_start(xdump.ap(), x[:])

        # ---- final LN + LM head phase (separate pools; trunk SBUF released)
        with tc.tile_pool(name="stat2", bufs=2) as stat2, \
             tc.tile_pool(name="sb2", bufs=2) as sb2, \
             tc.tile_pool(name="hd", bufs=1) as hd, \
             tc.tile_pool(name="emb", bufs=3) as epool, \
             tc.tile_pool(name="hout", bufs=4) as hout, \
             tc.tile_pool(name="psT2", bufs=1, space="PSUM") as psT2, \
             tc.tile_pool(name="psH", bufs=4, space="PSUM") as psH:
            if stages >= 9:
                xnf_fm = hd.tile([128, 8, TOK], BF16)
                _ln_tm(nc, sb2, stat2, psT2, x, xnf_fm, ident, eng_evac=0)
                # world AllGather split into token halves so head MMs for the
                # first half start while the second half is still in flight
                for hh in range(2):
                    nc.sync.dma_start(
                        agf_in[hh].ap().rearrange("p (kk t) -> p kk t", kk=8),
                        xnf_fm[:, :, hh * 256:(hh + 1) * 256])
                    if no_cc:
                        for r_ in range(N_CORES):
                            nc.sync.dma_start(
                                agf_out[hh][r_ * 128:(r_ + 1) * 128, :], agf_in[hh][:])
                    else:
                        nc.gpsimd.collective_compute(
                            "AllGather", OP.bypass, replica_groups=WORLD,
                            ins=[agf_in[hh][:]], outs=[agf_out[hh][:]])
                if dbg:
                    dbg_dump("xnf_fm", xnf_fm[:], [128, 8, TOK])
                xn_all = hd.tile([128, 64, TOK], BF16)
                for hh in range(2):
                    for r_ in range(8):
                        nc.sync.dma_start(
                            xn_all[:, r_ * 8:(r_ + 1) * 8, hh * 256:(hh + 1) * 256],
                            agf_out[hh][r_ * 128:(r_ + 1) * 128, :].rearrange(
                                "p (kk t) -> p kk t", kk=8))
                nchunks = [(i * 512, min(512, VP - i * 512)) for i in range(NVC)]
                for ni, (n0, nsz) in enumerate(nchunks):
                    et = epool.tile([128, 8, 512], BF16, tag="emb")
                    nc.sync.dma_start(et[:], embT_h[ni])
                    # token half 0 (tiles 0,1 of every rank) first: available
                    # as soon as the first half-AG lands
                    for mi in ([m for m in range(32) if m % 4 < 2]
                               + [m for m in range(32) if m % 4 >= 2]):
                        r, t = mi // 4, mi % 4
                        ps = psH.tile([128, nsz], F32, tag="h")
                        for kk in range(8):
                            nc.tensor.matmul(
                                ps[:], xn_all[:, r * 8 + kk, t * 128:(t + 1) * 128],
                                et[:, kk, 0:nsz],
                                start=(kk == 0), stop=(kk == 7))
                        osb = hout.tile([128, nsz], BF16, tag="o")
                        if mi % 2 == 0:
                            nc.vector.tensor_copy(osb[:], ps[:])
                        else:
                            nc.scalar.copy(osb[:], ps[:])
                        nc.sync.dma_start(out_h[mi * 128:(mi + 1) * 128, n0:n0 + nsz], osb[:])

    nc.compile()
    return nc, dbg_outs


def _fm_tile_w(w):
    """[1024, nch*512] -> [nch, 128, 8, 512]; tile[j,p,kk,c] = w[kk*128+p, j*512+c]."""
    din, dout = w.shape
    nch = dout // 512
    r = w.reshape(8, 128, nch, 512)
    return np.ascontiguousarray(r.transpose(2, 1, 0, 3))


def prepare_inputs(idx, tok_emb, pos_emb, qkv_w, qkv_b, proj_w, proj_b,
                   ff1_w, ff1_b, ff2_w, ff2_b, ln1_s, ln1_b, ln2_s, ln2_b,
                   lnf_s, lnf_b, n_layers=L):
    """Host-side sharding/folding. Returns per-core in_maps."""
    bf = ml_dtypes.bfloat16
    for name, v in (("qkv_b", qkv_b), ("proj_b", proj_b), ("ff1_b", ff1_b),
                    ("ff2_b", ff2_b), ("ln1_b", ln1_b), ("ln2_b", ln2_b),
                    ("lnf_b", lnf_b)):
        assert np.allclose(np.asarray(v), 0.0), f"nonzero {name} not supported"

    idx = np.asarray(idx)
    tok_emb = np.asarray(tok_emb, np.float32)
    pos_emb = np.asarray(pos_emb, np.float32)
    scale = 1.0 / np.sqrt(HD)

    # fold LN scales + attention scale into weights (exact)
    wqkv = (np.asarray(qkv_w[:n_layers], np.float32)
            * np.asarray(ln1_s[:n_layers], np.float32)[:, :, None]).copy()
    wqkv[:, :, :D] *= scale
    w1 = (np.asarray(ff1_w[:n_layers], np.float32)
          * np.asarray(ln2_s[:n_layers], np.float32)[:, :, None])
    wp = np.asarray(proj_w[:n_layers], np.float32)
    w2 = np.asarray(ff2_w[:n_layers], np.float32)
    embT_full = (tok_emb * np.asarray(lnf_s, np.float32)[None, :]).T  # [D, V]
    embT_pad = np.zeros((D, N_CORES * VP), np.float32)
    embT_pad[:, :V] = embT_full

    # pre-tiled weight arrays (contiguous 1MB DMA bursts on device)
    wqkv_t = np.stack([_fm_tile_w(wqkv[l]) for l in range(n_layers)]).astype(bf)
    wp_t = np.stack([_fm_tile_w(wp[l]) for l in range(n_layers)]).astype(bf)
    w1_t = np.stack([_fm_tile_w(w1[l]) for l in range(n_layers)]).astype(bf)
    # w2: [4096, 1024] -> [2(nch), 8(kkc), 128(p), 4(kki), 512(c)]
    w2_t = np.stack([
        np.ascontiguousarray(
            w2[l].reshape(8, 4, 128, 2, 512).transpose(3, 0, 2, 1, 4))
        for l in range(n_layers)]).astype(bf)

    ident = np.eye(128, dtype=bf)
    ones64 = np.ones((1, 64), bf)

    tri = np.tril(np.ones((128, 128), np.float32)).T  # [kt, q] valid kt<=q
    # core-relative banks: slot0 = local diagonal (triangular for both
    # ranks); slot1 = remote diagonal (all-masked for r=0, visible for r=1)
    msk_r = [np.zeros((2, 128, 128), np.float32) for _ in range(2)]
    msk_r[0][0] = tri
    msk_r[0][1] = 0.0
    msk_r[1][0] = tri
    msk_r[1][1] = 1.0

    in_maps = []
    for c in range(N_CORES):
        b, r = c // 2, c % 2
        pos = positions_for_rank(r)
        x0 = tok_emb[idx[b, pos]] + pos_emb[pos]
        # per-core vocab slice, padded to 13*512 cols for uniform DMA
        esl = np.zeros((D, NVC * 512), np.float32)
        esl[:, :VP] = embT_pad[:, c * VP:(c + 1) * VP]
        embT_tiles = np.ascontiguousarray(
            esl.reshape(8, 128, NVC, 512).transpose(2, 1, 0, 3)).astype(bf)
        in_maps.append({
            "x0": np.ascontiguousarray(x0, np.float32),
            "wqkv": wqkv_t, "wp": wp_t, "w1": w1_t, "w2": w2_t,
            "embT": embT_tiles,
            "msk": msk_r[r].astype(bf),
            "identin": ident,
            "ones64": ones64,
        })
    return in_maps


def assemble_output(results):
    """Per-core [4096, VP] bf16 -> full logits [B, T, V] f32."""
    logits = np.empty((B, T, V), np.float32)
    pos_r = [positions_for_rank(0), positions_for_rank(1)]
    for c in range(N_CORES):
        out_c = np.asarray(results[c]["out"], np.float32)  # [4096, VP]
        v0 = c * VP
        ncols = min(VP, V - v0)
        if ncols <= 0:
            continue
        for r in range(N_CORES):
            bb, rr = r // 2, r % 2
            logits[bb, pos_r[rr], v0:v0 + ncols] = \
                out_c[r * TOK:(r + 1) * TOK, :ncols]
    return logits


_NC_CACHE = {}


def _get_nc(n_layers=L, dbg=False):
    key = (n_layers, dbg)
    if key not in _NC_CACHE:
        _NC_CACHE[key] = build(n_layers=n_layers, dbg=dbg)
    return _NC_CACHE[key]


def kernel(**inputs):
    in_maps = prepare_inputs(**inputs)
    nc, _ = _get_nc()
    res = run_bass_kernel_spmd(nc, in_maps, core_ids=list(range(N_CORES)))
    return assemble_output(res.results)


# revision 27
# speedup vs baseline: 1.0444x; 1.0399x over previous
"""Trainium2 Bass kernel for nn_AtomsGPT (GPT-2-style dense transformer).

B=4, T=1024, D=1024, H=16 heads, L=8 layers, V=50257, tied LM head.

Sharding (8 NeuronCores):
- Token-data-parallel trunk: core c owns batch c//2, pair-rank r=c%2.
  Rank r takes the even (r=0) / odd (r=1) 128-position tiles of the
  sequence, interleaved for causal-attention load balance.
- Per layer the pair exchanges LN1 outputs via a 2-rank AllGather.  The
  local K/V matmuls run from the local xn while the AllGather is in
  flight; the remote half is recovered bit-exactly as
  (bank0+bank1) - xn_local in fp32, keeping the program rank-agnostic
  (k_all/v_all banks are core-relative: bank0=local, bank1=remote).
- Attention processes head PAIRS: the two 64-wide score matmuls use
  disjoint PE row groups (tile_position auto-derived) and run
  concurrently.  Causal masks are per-core input data (local diagonal =
  triangular for both ranks; remote diagonal = all-ones / all-zeros).
- The tied LM head is sharded over vocab (6288 cols/core); embedding
  weights stream chunk-by-chunk (no SBUF preload); the final world
  AllGather is split into token halves so head matmuls start early.

All matmuls run in bf16 with fp32 PSUM accumulation; the residual stream
and layernorm statistics stay fp32. LN scales and the attention scale
are folded into weight matrices on the host (exact); all bias vectors in
this problem are structurally zero (asserted).
"""

import sys

for _p in ("/opt/trn_rl_repo", "/root/.axon_site"):
    if _p not in sys.path:
        sys.path.insert(0, _p)

import numpy as np
import ml_dtypes

import concourse.bass as bass
import concourse.tile as tile
from concourse import bacc, mybir
from concourse.bass_utils import run_bass_kernel_spmd

F32 = mybir.dt.float32
BF16 = mybir.dt.bfloat16
AF = mybir.ActivationFunctionType
OP = mybir.AluOpType

B, T, D, H, L, V = 4, 1024, 1024, 16, 8, 50257
HD = D // H  # 64
EPS = 1e-5
N_CORES = 8
TOK = 512           # tokens per core
P = 128
VP = 6288           # per-core padded vocab slice (8*6288 = 50304 >= V)
NVC = (VP + 511) // 512  # vocab chunks (13; last is 144 wide)
PAIRS = [[0, 1], [2, 3], [4, 5], [6, 7]]
WORLD = [list(range(N_CORES))]
DMA_TRANSPOSE = False


def positions_for_rank(r):
    """Global positions owned by pair-rank r, in local order (increasing)."""
    tiles = [2 * j + r for j in range(4)]
    return np.concatenate([np.arange(128 * t, 128 * (t + 1)) for t in tiles])


def _ln_tm(nc, sb, stat, psT, x_ap, xn_fm, ident, eng_evac):
    """LayerNorm of token-major x_ap [128, 4, 1024] f32 -> feature-major
    bf16 xn_fm [128, 8, 512].  Scale/bias are folded into downstream
    weights on the host, so this computes plain (x - mean) * rsqrt(var)."""
    ssum = stat.tile([128, 4], F32, tag="ssum")
    ssq = stat.tile([128, 4], F32, tag="ssq")
    for t in range(4):
        nc.vector.reduce_sum(ssum[:, t : t + 1], x_ap[:, t, :], axis=mybir.AxisListType.X)
        sc = sb.tile([128, 1024], F32, tag="ln_sc")
        nc.scalar.activation(sc[:], x_ap[:, t, :], AF.Square,
                             accum_out=ssq[:, t : t + 1])
    m = stat.tile([128, 4], F32, tag="m")
    nc.vector.tensor_scalar_mul(m[:], ssum[:], 1.0 / D)
    var = stat.tile([128, 4], F32, tag="var")
    nc.vector.tensor_scalar_mul(var[:], ssq[:], 1.0 / D)
    mm = stat.tile([128, 4], F32, tag="mm")
    nc.vector.tensor_mul(mm[:], m[:], m[:])
    nc.vector.tensor_sub(var[:], var[:], mm[:])
    eps = stat.tile([128, 1], F32, tag="eps")
    nc.vector.memset(eps[:], EPS)
    std = stat.tile([128, 4], F32, tag="std")
    nc.scalar.activation(std[:], var[:], AF.Sqrt, bias=eps[:])
    rstd = stat.tile([128, 4], F32, tag="rstd")
    nc.vector.reciprocal(rstd[:], std[:])
    nmr = stat.tile([128, 4], F32, tag="nmr")
    nc.vector.tensor_mul(nmr[:], m[:], rstd[:])
    nc.vector.tensor_scalar_mul(nmr[:], nmr[:], -1.0)
    for t in range(4):
        xn = sb.tile([128, 1024], BF16, tag="ln_xn")
        nc.vector.tensor_scalar(xn[:], x_ap[:, t, :], rstd[:, t : t + 1],
                                nmr[:, t : t + 1], OP.mult, OP.add)
        if DMA_TRANSPOSE:
            for kk in range(8):
                nc.sync.dma_start_transpose(
                    xn_fm[:, kk, t * 128:(t + 1) * 128],
                    xn[:, kk * 128:(kk + 1) * 128])
        else:
            ptr = psT.tile([128, 8, 128], BF16, tag="tr")
            for kk in range(8):
                nc.tensor.transpose(ptr[:, kk, :], xn[:, kk * 128:(kk + 1) * 128], ident[:])
            eng = nc.vector if (eng_evac + t) % 2 == 0 else nc.scalar
            if eng is nc.vector:
                nc.vector.tensor_copy(xn_fm[:, :, t * 128:(t + 1) * 128], ptr[:])
            else:
                nc.scalar.copy(xn_fm[:, :, t * 128:(t + 1) * 128], ptr[:])


def build(n_layers=L, dbg=False, no_cc=False, stages=99):
    nc = bacc.Bacc("TRN2", target_bir_lowering=False, debug=False,
                   num_devices=N_CORES)

    x0_h = nc.dram_tensor("x0", [TOK, D], F32, kind="ExternalInput")
    # pre-tiled weights: [chunks, 128(p), 8(kk), 512(c)] per layer
    wqkv_h = nc.dram_tensor("wqkv", [n_layers, 6, 128, 8, 512], BF16, kind="ExternalInput")
    wp_h = nc.dram_tensor("wp", [n_layers, 2, 128, 8, 512], BF16, kind="ExternalInput")
    w1_h = nc.dram_tensor("w1", [n_layers, 8, 128, 8, 512], BF16, kind="ExternalInput")
    w2_h = nc.dram_tensor("w2", [n_layers, 2, 8, 128, 4, 512], BF16, kind="ExternalInput")
    embT_h = nc.dram_tensor("embT", [NVC, 128, 8, 512], BF16, kind="ExternalInput")
    msk_h = nc.dram_tensor("msk", [2, 128, 128], BF16, kind="ExternalInput")
    ident_h = nc.dram_tensor("identin", [128, 128], BF16, kind="ExternalInput")
    ones64_h = nc.dram_tensor("ones64", [1, 64], BF16, kind="ExternalInput")
    out_h = nc.dram_tensor("out", [N_CORES * TOK, VP], BF16, kind="ExternalOutput")

    dbg_outs = {}

    def dbg_dump(name, ap, shape, rearr=None):
        if not dbg:
            return
        t = nc.dram_tensor(f"dbg_{name}", list(shape), ap.dtype, kind="ExternalOutput")
        dst = t.ap() if rearr is None else t.ap().rearrange(rearr)
        nc.sync.dma_start(dst, ap)
        dbg_outs[name] = shape

    ag_in = [nc.dram_tensor(f"agin{l}", [128, 4096], BF16, kind="Internal")
             for l in range(n_layers)]
    ag_out = [nc.dram_tensor(f"agout{l}", [256, 4096], BF16, kind="Internal")
              for l in range(n_layers)]
    # final AG split into token halves: [128, 8(kk) x 256(tok-half)]
    agf_in = [nc.dram_tensor(f"agfin{h}", [128, 2048], BF16, kind="Internal")
              for h in range(2)]
    agf_out = [nc.dram_tensor(f"agfout{h}", [N_CORES * 128, 2048], BF16,
                              kind="Internal", addr_space="Shared")
               for h in range(2)]

    with tile.TileContext(nc) as tc:
      with tc.tile_pool(name="const", bufs=1) as constp, \
           tc.tile_pool(name="xres", bufs=1) as xresp:
        with tc.tile_pool(name="stat", bufs=2) as stat, \
             tc.tile_pool(name="sb", bufs=2) as sb, \
             tc.tile_pool(name="act", bufs=1) as actp, \
             tc.tile_pool(name="wch", bufs=6) as wch, \
             tc.tile_pool(name="pp", bufs=8) as pp, \
             tc.tile_pool(name="psA", bufs=4, space="PSUM") as psA, \
             tc.tile_pool(name="psB", bufs=3, space="PSUM") as psB, \
             tc.tile_pool(name="psT", bufs=1, space="PSUM") as psT:

            ident = constp.tile([128, 128], BF16)
            nc.sync.dma_start(ident[:], ident_h[:])
            msk = constp.tile([128, 2, 128], BF16)
            nc.sync.dma_start(msk[:], msk_h.ap().rearrange("b p q -> p b q"))
            ones64 = constp.tile([1, 64], BF16)
            nc.sync.dma_start(ones64[:], ones64_h[:])

            # residual stream, token-major fp32 [part, tok-tile, D]
            x = xresp.tile([128, 4, D], F32)
            nc.sync.dma_start(x[:], x0_h.ap().rearrange("(t p) d -> p t d", p=128))

            for l in range(n_layers):
                # ---- LN1 -> xn_fm (feature-major bf16), kick pair AllGather
                xn_fm = actp.tile([128, 8, TOK], BF16, tag="xn_fm")
                _ln_tm(nc, sb, stat, psT, x, xn_fm, ident, eng_evac=0)
                nc.sync.dma_start(
                    ag_in[l].ap().rearrange("p (kk t) -> p kk t", kk=8), xn_fm[:])
                if no_cc:
                    nc.sync.dma_start(ag_out[l][0:128, :], ag_in[l][:])
                    nc.sync.dma_start(ag_out[l][128:256, :], ag_in[l][:])
                else:
                    nc.gpsimd.collective_compute(
                        "AllGather", OP.bypass, replica_groups=PAIRS,
                        ins=[ag_in[l][:]], outs=[ag_out[l][:]])
                if l == 0:
                    dbg_dump("xn_fm0", xn_fm[:], [128, 8, TOK])

                # ---- Q^T (feature-major) from local xn_fm (covers the AG)
                if stages < 3:
                    continue
                q_fm = actp.tile([128, 8, TOK], BF16, tag="q_fm")
                for ch in range(2):
                    wt = wch.tile([128, 8, 512], BF16, tag="w", name=f"wq{l}_{ch}")
                    nc.sync.dma_start(wt[:], wqkv_h[l, ch])
                    for mi in range(4):
                        ps = psA.tile([128, TOK], F32, tag="mm")
                        for kk in range(8):
                            nc.tensor.matmul(ps[:], wt[:, kk, mi * 128:(mi + 1) * 128],
                                             xn_fm[:, kk, :], start=(kk == 0), stop=(kk == 7))
                        nc.scalar.copy(q_fm[:, ch * 4 + mi, :], ps[:])

                # ---- K/V for LOCAL tokens (bank 0) — also covers the AG
                k_all = actp.tile([128, 16, TOK], BF16, tag="k_all")
                wkt = {}
                for ch in range(2):
                    wt = wch.tile([128, 8, 512], BF16, tag="w", name=f"wk{l}_{ch}")
                    nc.sync.dma_start(wt[:], wqkv_h[l, 2 + ch])
                    wkt[ch] = wt
                    for mi in range(4):
                        ps = psA.tile([128, TOK], F32, tag="mm")
                        for kk in range(8):
                            nc.tensor.matmul(
                                ps[:], wt[:, kk, mi * 128:(mi + 1) * 128],
                                xn_fm[:, kk, :], start=(kk == 0), stop=(kk == 7))
                        nc.vector.tensor_copy(k_all[:, ch * 4 + mi, :], ps[:])

                v_all = actp.tile([128, 8, H, HD + 1], BF16, tag="v_all")
                nc.vector.memset(v_all[:, :, :, HD:HD + 1], 1.0)
                wvt = {}
                for ch in range(2):
                    wt = wch.tile([128, 8, 512], BF16, tag="w", name=f"wv{l}_{ch}")
                    nc.sync.dma_start(wt[:], wqkv_h[l, 4 + ch])
                    wvt[ch] = wt
                    for t in range(4):
                        ps = psA.tile([128, TOK], F32, tag="mm")
                        for kk in range(8):
                            nc.tensor.matmul(
                                ps[:], xn_fm[:, kk, t * 128:(t + 1) * 128],
                                wt[:, kk, :], start=(kk == 0), stop=(kk == 7))
                        nc.vector.tensor_copy(
                            v_all[:, t, ch * 8:(ch + 1) * 8, 0:HD],
                            ps[:].rearrange("p (h d) -> p h d", h=8))

                # ---- gather pair xn; recover remote half bit-exactly as
                # (bank0 + bank1) - xn_local (fp32 intermediate => exact)
                if stages < 4:
                    continue
                xn_rem = actp.tile([128, 8, TOK], BF16, tag="xn_rem")
                for kk in range(8):
                    xb0 = sb.tile([128, TOK], BF16, tag="xb0")
                    xb1 = sb.tile([128, TOK], BF16, tag="xb1")
                    nc.sync.dma_start(xb0[:], ag_out[l][0:128, kk * TOK:(kk + 1) * TOK])
                    nc.sync.dma_start(xb1[:], ag_out[l][128:256, kk * TOK:(kk + 1) * TOK])
                    tmp = sb.tile([128, TOK], F32, tag="tmp32")
                    nc.vector.tensor_add(tmp[:], xb0[:], xb1[:])
                    nc.vector.tensor_sub(xn_rem[:, kk, :], tmp[:], xn_fm[:, kk, :])

                # ---- remote K/V and attention, interleaved in groups of two
                # head pairs: the scalar-bound exp stream of one group
                # overlaps the PE-bound remote-K/V matmuls of the next
                if stages < 5:
                    continue
                o_fm = actp.tile([128, 8, TOK], BF16, tag="o_fm")

                def attn_pair(hp):
                    he, ho = 2 * hp, 2 * hp + 1
                    ave = psB.tile([P, TOK], F32, tag="acc", name=f"av{l}_{he}")
                    avo = psB.tile([P, TOK], F32, tag="acc", name=f"av{l}_{ho}")
                    for b in range(2):
                        for i in range(4):
                            n = TOK - 128 * i
                            spe = psA.tile([128, n], F32, tag="mm")
                            spo = psA.tile([128, n], F32, tag="mm")
                            nc.tensor.matmul(
                                spe[:], k_all[0:64, b * 8 + hp, i * 128:(i + 1) * 128],
                                q_fm[0:64, hp, 128 * i:TOK], start=True, stop=True)
                            nc.tensor.matmul(
                                spo[:], k_all[64:128, b * 8 + hp, i * 128:(i + 1) * 128],
                                q_fm[64:128, hp, 128 * i:TOK], start=True, stop=True)
                            pte = pp.tile([128, n], BF16, tag="p")
                            pto = pp.tile([128, n], BF16, tag="p")
                            nc.scalar.activation(pte[:], spe[:], AF.Exp)
                            nc.scalar.activation(pto[:], spo[:], AF.Exp)
                            # causal mask on the diagonal 128-col block
                            nc.vector.tensor_mul(pte[:, 0:128], pte[:, 0:128], msk[:, b, :])
                            nc.vector.tensor_mul(pto[:, 0:128], pto[:, 0:128], msk[:, b, :])
                            nc.tensor.matmul(
                                ave[0:HD + 1, 128 * i:TOK],
                                v_all[:, b * 4 + i, he, :], pte[:],
                                start=(b == 0 and i == 0), stop=(b == 1 and i == 3))
                            nc.tensor.matmul(
                                avo[0:HD + 1, 128 * i:TOK],
                                v_all[:, b * 4 + i, ho, :], pto[:],
                                start=(b == 0 and i == 0), stop=(b == 1 and i == 3))
                    # denominators -> broadcast (ones64 MMs) -> fast approx
                    # reciprocal (~18 bits, plenty vs bf16 downstream)
                    den = sb.tile([1, 2, TOK], BF16, tag="den")
                    nc.vector.tensor_copy(den[0:1, 0, :], ave[HD:HD + 1, :])
                    nc.vector.tensor_copy(den[0:1, 1, :], avo[HD:HD + 1, :])
                    bp = psA.tile([128, TOK], F32, tag="mm")
                    nc.tensor.matmul(bp[0:64, :], ones64[:], den[0:1, 0, :],
                                     start=True, stop=True)
                    nc.tensor.matmul(bp[64:128, :], ones64[:], den[0:1, 1, :],
                                     start=True, stop=True)
                    rb = sb.tile([128, TOK], F32, tag="rb")
                    nc.vector.reciprocal_approx_fast(rb[:], bp[:])
                    nc.vector.tensor_tensor(o_fm[0:64, hp, :], ave[0:HD, :],
                                            rb[0:64, :], OP.mult)
                    nc.vector.tensor_tensor(o_fm[64:128, hp, :], avo[0:HD, :],
                                            rb[64:128, :], OP.mult)

                # remote K/V first (dense PE stream), attention after: the PE
                # queue is strict FIFO, so attention's exp-gated AV matmuls
                # must not sit in front of independent dense work
                for ch in range(2):
                    for mi in range(4):
                        ps = psA.tile([128, TOK], F32, tag="mm")
                        for kk in range(8):
                            nc.tensor.matmul(
                                ps[:], wkt[ch][:, kk, mi * 128:(mi + 1) * 128],
                                xn_rem[:, kk, :], start=(kk == 0), stop=(kk == 7))
                        nc.vector.tensor_copy(k_all[:, 8 + ch * 4 + mi, :], ps[:])
                for ch in range(2):
                    for t in range(4):
                        ps = psA.tile([128, TOK], F32, tag="mm")
                        for kk in range(8):
                            nc.tensor.matmul(
                                ps[:], xn_rem[:, kk, t * 128:(t + 1) * 128],
                                wvt[ch][:, kk, :], start=(kk == 0), stop=(kk == 7))
                        nc.vector.tensor_copy(
                            v_all[:, 4 + t, ch * 8:(ch + 1) * 8, 0:HD],
                            ps[:].rearrange("p (h d) -> p h d", h=8))
                for hp in range(H // 2):
                    attn_pair(hp)
                if l == 0:
                    dbg_dump("k_all0", k_all[:], [128, 16, TOK])
                    dbg_dump("v_all0", v_all[:], [128, 8, H, HD + 1])
                    dbg_dump("o_fm0", o_fm[:], [128, 8, TOK])

                # ---- projection (token-major) + residual
                if stages < 7:
                    continue
                for ch in range(2):
                    wt = wch.tile([128, 8, 512], BF16, tag="w", name=f"wpj{l}_{ch}")
                    nc.sync.dma_start(wt[:], wp_h[l, ch])
                    for t in range(4):
                        ps = psA.tile([128, 512], F32, tag="mm")
                        for kk in range(8):
                            nc.tensor.matmul(
                                ps[:], o_fm[:, kk, t * 128:(t + 1) * 128],
                                wt[:, kk, :], start=(kk == 0), stop=(kk == 7))
                        nc.vector.tensor_add(x[:, t, ch * 512:(ch + 1) * 512],
                                             x[:, t, ch * 512:(ch + 1) * 512], ps[:])
                if l == 0:
                    dbg_dump("xattn0", x[:], [128, 4, D])

                # ---- LN2 -> xn2_fm
                if stages < 8:
                    continue
                xn2_fm = actp.tile([128, 8, TOK], BF16, tag="xn2_fm")
                _ln_tm(nc, sb, stat, psT, x, xn2_fm, ident, eng_evac=1)

                # ---- FFN: ff1 full-token, ff2 in token halves
                h_sb = actp.tile([128, 32, TOK], BF16, tag="h_sb")
                for mc in range(8):
                    wt = wch.tile([128, 8, 512], BF16, tag="w", name=f"w1_{l}_{mc}")
                    nc.sync.dma_start(wt[:], w1_h[l, mc])
                    for mi in range(4):
                        ps = psA.tile([128, TOK], F32, tag="mm")
                        for kk in range(8):
                            nc.tensor.matmul(
                                ps[:], wt[:, kk, mi * 128:(mi + 1) * 128],
                                xn2_fm[:, kk, :], start=(kk == 0), stop=(kk == 7))
                        nc.scalar.activation(h_sb[:, mc * 4 + mi, :], ps[:], AF.Gelu)
                for half in range(2):
                    for nch in range(2):
                        acc = [psB.tile([128, 512], F32, tag="acc",
                                        name=f"acc{l}_{half}_{nch}_{a}") for a in range(2)]
                        for kkc in range(8):
                            w2t = wch.tile([128, 4, 512], BF16, tag="w",
                                           name=f"w2_{l}_{half}_{nch}_{kkc}")
                            nc.sync.dma_start(w2t[:], w2_h[l, nch, kkc])
                            for kki in range(4):
                                for mi in range(2):
                                    nc.tensor.matmul(
                                        acc[mi][:],
                                        h_sb[:, kkc * 4 + kki,
                                             half * 256 + mi * 128:half * 256 + (mi + 1) * 128],
                                        w2t[:, kki, :],
                                        start=(kkc == 0 and kki == 0),
                                        stop=(kkc == 7 and kki == 3))
                        for mi in range(2):
                            t = half * 2 + mi
                            nc.vector.tensor_add(x[:, t, nch * 512:(nch + 1) * 512],
                                                 x[:, t, nch * 512:(nch + 1) * 512],
                                                 acc[mi][:])
                if l == 0:
                    dbg_dump("xlayer0", x[:], [128, 4, D])

            if stages < 9:
                # early-exit build for bisection: dump residual so work isn't DCE'd
                xdump = nc.dram_tensor("xdump", [128, 4, D], F32, kind="ExternalOutput")
                nc.sync.dma_start(xdump.ap(), x[:])

        # ---- final LN + LM head phase (separate pools; trunk SBUF released)
        with tc.tile_pool(name="stat2", bufs=2) as stat2, \
             tc.tile_pool(name="sb2", bufs=2) as sb2, \
             tc.tile_pool(name="hd", bufs=1) as hd, \
             tc.tile_pool(name="emb", bufs=3) as epool, \
             tc.tile_pool(name="hout", bufs=4) as hout, \
             tc.tile_pool(name="psT2", bufs=1, space="PSUM") as psT2, \
             tc.tile_pool(name="psH", bufs=4, space="PSUM") as psH:
            if stages >= 9:
                xnf_fm = hd.tile([128, 8, TOK], BF16)
                _ln_tm(nc, sb2, stat2, psT2, x, xnf_fm, ident, eng_evac=0)
                # world AllGather split into token halves so head MMs for the
                # first half start while the second half is still in flight
                for hh in range(2):
                    nc.sync.dma_start(
                        agf_in[hh].ap().rearrange("p (kk t) -> p kk t", kk=8),
                        xnf_fm[:, :, hh * 256:(hh + 1) * 256])
                    if no_cc:
                        for r_ in range(N_CORES):
                            nc.sync.dma_start(
                                agf_out[hh][r_ * 128:(r_ + 1) * 128, :], agf_in[hh][:])
                    else:
                        nc.gpsimd.collective_compute(
                            "AllGather", OP.bypass, replica_groups=WORLD,
                            ins=[agf_in[hh][:]], outs=[agf_out[hh][:]])
                if dbg:
                    dbg_dump("xnf_fm", xnf_fm[:], [128, 8, TOK])
                xn_all = hd.tile([128, 64, TOK], BF16)
                for hh in range(2):
                    for r_ in range(8):
                        nc.sync.dma_start(
                            xn_all[:, r_ * 8:(r_ + 1) * 8, hh * 256:(hh + 1) * 256],
                            agf_out[hh][r_ * 128:(r_ + 1) * 128, :].rearrange(
                                "p (kk t) -> p kk t", kk=8))
                nchunks = [(i * 512, min(512, VP - i * 512)) for i in range(NVC)]
                for ni, (n0, nsz) in enumerate(nchunks):
                    et = epool.tile([128, 8, 512], BF16, tag="emb")
                    nc.sync.dma_start(et[:], embT_h[ni])
                    # token half 0 (tiles 0,1 of every rank) first: available
                    # as soon as the first half-AG lands
                    for mi in ([m for m in range(32) if m % 4 < 2]
                               + [m for m in range(32) if m % 4 >= 2]):
                        r, t = mi // 4, mi % 4
                        ps = psH.tile([128, nsz], F32, tag="h")
                        for kk in range(8):
                            nc.tensor.matmul(
                                ps[:], xn_all[:, r * 8 + kk, t * 128:(t + 1) * 128],
                                et[:, kk, 0:nsz],
                                start=(kk == 0), stop=(kk == 7))
                        osb = hout.tile([128, nsz], BF16, tag="o")
                        if mi % 2 == 0:
                            nc.vector.tensor_copy(osb[:], ps[:])
                        else:
                            nc.scalar.copy(osb[:], ps[:])
                        nc.sync.dma_start(out_h[mi * 128:(mi + 1) * 128, n0:n0 + nsz], osb[:])

    nc.compile()
    return nc, dbg_outs


def _fm_tile_w(w):
    """[1024, nch*512] -> [nch, 128, 8, 512]; tile[j,p,kk,c] = w[kk*128+p, j*512+c]."""
    din, dout = w.shape
    nch = dout // 512
    r = w.reshape(8, 128, nch, 512)
    return np.ascontiguousarray(r.transpose(2, 1, 0, 3))


def prepare_inputs(idx, tok_emb, pos_emb, qkv_w, qkv_b, proj_w, proj_b,
                   ff1_w, ff1_b, ff2_w, ff2_b, ln1_s, ln1_b, ln2_s, ln2_b,
                   lnf_s, lnf_b, n_layers=L):
    """Host-side sharding/folding. Returns per-core in_maps."""
    bf = ml_dtypes.bfloat16
    for name, v in (("qkv_b", qkv_b), ("proj_b", proj_b), ("ff1_b", ff1_b),
                    ("ff2_b", ff2_b), ("ln1_b", ln1_b), ("ln2_b", ln2_b),
                    ("lnf_b", lnf_b)):
        assert np.allclose(np.asarray(v), 0.0), f"nonzero {name} not supported"

    idx = np.asarray(idx)
    tok_emb = np.asarray(tok_emb, np.float32)
    pos_emb = np.asarray(pos_emb, np.float32)
    scale = 1.0 / np.sqrt(HD)

    # fold LN scales + attention scale into weights (exact)
    wqkv = (np.asarray(qkv_w[:n_layers], np.float32)
            * np.asarray(ln1_s[:n_layers], np.float32)[:, :, None]).copy()
    wqkv[:, :, :D] *= scale
    w1 = (np.asarray(ff1_w[:n_layers], np.float32)
          * np.asarray(ln2_s[:n_layers], np.float32)[:, :, None])
    wp = np.asarray(proj_w[:n_layers], np.float32)
    w2 = np.asarray(ff2_w[:n_layers], np.float32)
    embT_full = (tok_emb * np.asarray(lnf_s, np.float32)[None, :]).T  # [D, V]
    embT_pad = np.zeros((D, N_CORES * VP), np.float32)
    embT_pad[:, :V] = embT_full

    # pre-tiled weight arrays (contiguous 1MB DMA bursts on device)
    wqkv_t = np.stack([_fm_tile_w(wqkv[l]) for l in range(n_layers)]).astype(bf)
    wp_t = np.stack([_fm_tile_w(wp[l]) for l in range(n_layers)]).astype(bf)
    w1_t = np.stack([_fm_tile_w(w1[l]) for l in range(n_layers)]).astype(bf)
    # w2: [4096, 1024] -> [2(nch), 8(kkc), 128(p), 4(kki), 512(c)]
    w2_t = np.stack([
        np.ascontiguousarray(
            w2[l].reshape(8, 4, 128, 2, 512).transpose(3, 0, 2, 1, 4))
        for l in range(n_layers)]).astype(bf)

    ident = np.eye(128, dtype=bf)
    ones64 = np.ones((1, 64), bf)

    tri = np.tril(np.ones((128, 128), np.float32)).T  # [kt, q] valid kt<=q
    # core-relative banks: slot0 = local diagonal (triangular for both
    # ranks); slot1 = remote diagonal (all-masked for r=0, visible for r=1)
    msk_r = [np.zeros((2, 128, 128), np.float32) for _ in range(2)]
    msk_r[0][0] = tri
    msk_r[0][1] = 0.0
    msk_r[1][0] = tri
    msk_r[1][1] = 1.0

    in_maps = []
    for c in range(N_CORES):
        b, r = c // 2, c % 2
        pos = positions_for_rank(r)
        x0 = tok_emb[idx[b, pos]] + pos_emb[pos]
        # per-core vocab slice, padded to 13*512 cols for uniform DMA
        esl = np.zeros((D, NVC * 512), np.float32)
        esl[:, :VP] = embT_pad[:, c * VP:(c + 1) * VP]
        embT_tiles = np.ascontiguousarray(
            esl.reshape(8, 128, NVC, 512).transpose(2, 1, 0, 3)).astype(bf)
        in_maps.append({
            "x0": np.ascontiguousarray(x0, np.float32),
            "wqkv": wqkv_t, "wp": wp_t, "w1": w1_t, "w2": w2_t,
            "embT": embT_tiles,
            "msk": msk_r[r].astype(bf),
            "identin": ident,
            "ones64": ones64,
        })
    return in_maps


def assemble_output(results):
    """Per-core [4096, VP] bf16 -> full logits [B, T, V] f32."""
    logits = np.empty((B, T, V), np.float32)
    pos_r = [positions_for_rank(0), positions_for_rank(1)]
    for c in range(N_CORES):
        out_c = np.asarray(results[c]["out"], np.float32)  # [4096, VP]
        v0 = c * VP
        ncols = min(VP, V - v0)
        if ncols <= 0:
            continue
        for r in range(N_CORES):
            bb, rr = r // 2, r % 2
            logits[bb, pos_r[rr], v0:v0 + ncols] = \
                out_c[r * TOK:(r + 1) * TOK, :ncols]
    return logits


_NC_CACHE = {}


def _get_nc(n_layers=L, dbg=False):
    key = (n_layers, dbg)
    if key not in _NC_CACHE:
        _NC_CACHE[key] = build(n_layers=n_layers, dbg=dbg)
    return _NC_CACHE[key]


def kernel(**inputs):
    in_maps = prepare_inputs(**inputs)
    nc, _ = _get_nc()
    res = run_bass_kernel_spmd(nc, in_maps, core_ids=list(range(N_CORES)))
    return assemble_output(res.results)


# revision 28
# speedup vs baseline: 1.0740x; 1.0283x over previous
"""Trainium2 Bass kernel for nn_AtomsGPT (GPT-2-style dense transformer).

B=4, T=1024, D=1024, H=16 heads, L=8 layers, V=50257, tied LM head.

Sharding (8 NeuronCores):
- Token-data-parallel trunk: core c owns batch c//2, pair-rank r=c%2.
  Rank r takes the even (r=0) / odd (r=1) 128-position tiles of the
  sequence, interleaved for causal-attention load balance.
- Per layer the pair exchanges LN1 outputs via a 2-rank AllGather.  The
  local K/V matmuls run from the local xn while the AllGather is in
  flight; the remote half is recovered bit-exactly as
  (bank0+bank1) - xn_local in fp32, keeping the program rank-agnostic
  (k_all/v_all banks are core-relative: bank0=local, bank1=remote).
- Attention processes head PAIRS: the two 64-wide score matmuls use
  disjoint PE row groups (tile_position auto-derived) and run
  concurrently.  Causal masks are per-core input data (local diagonal =
  triangular for both ranks; remote diagonal = all-ones / all-zeros).
- The tied LM head is sharded over vocab (6288 cols/core); embedding
  weights stream chunk-by-chunk (no SBUF preload); the final world
  AllGather is split into token halves so head matmuls start early.

All matmuls run in bf16 with fp32 PSUM accumulation; the residual stream
and layernorm statistics stay fp32. LN scales and the attention scale
are folded into weight matrices on the host (exact); all bias vectors in
this problem are structurally zero (asserted).
"""

import sys

for _p in ("/opt/trn_rl_repo", "/root/.axon_site"):
    if _p not in sys.path:
        sys.path.insert(0, _p)

import numpy as np
import ml_dtypes

import concourse.bass as bass
import concourse.tile as tile
from concourse import bacc, mybir
from concourse.bass_utils import run_bass_kernel_spmd

F32 = mybir.dt.float32
BF16 = mybir.dt.bfloat16
AF = mybir.ActivationFunctionType
OP = mybir.AluOpType

B, T, D, H, L, V = 4, 1024, 1024, 16, 8, 50257
HD = D // H  # 64
EPS = 1e-5
N_CORES = 8
TOK = 512           # tokens per core
P = 128
VP = 6288           # per-core padded vocab slice (8*6288 = 50304 >= V)
NVC = (VP + 511) // 512  # vocab chunks (13; last is 144 wide)
PAIRS = [[0, 1], [2, 3], [4, 5], [6, 7]]
WORLD = [list(range(N_CORES))]
DMA_TRANSPOSE = False


def positions_for_rank(r):
    """Global positions owned by pair-rank r, in local order (increasing)."""
    tiles = [2 * j + r for j in range(4)]
    return np.concatenate([np.arange(128 * t, 128 * (t + 1)) for t in tiles])


def _ln_tm(nc, sb, stat, psT, x_ap, xn_fm, ident, eng_evac):
    """LayerNorm of token-major x_ap [128, 4, 1024] f32 -> feature-major
    bf16 xn_fm [128, 8, 512].  Scale/bias are folded into downstream
    weights on the host, so this computes plain (x - mean) * rsqrt(var)."""
    ssum = stat.tile([128, 4], F32, tag="ssum")
    ssq = stat.tile([128, 4], F32, tag="ssq")
    m = stat.tile([128, 4], F32, tag="m")
    var = stat.tile([128, 4], F32, tag="var")
    mm = stat.tile([128, 4], F32, tag="mm")
    std = stat.tile([128, 4], F32, tag="std")
    rstd = stat.tile([128, 4], F32, tag="rstd")
    nmr = stat.tile([128, 4], F32, tag="nmr")
    eps = stat.tile([128, 1], F32, tag="eps")
    nc.vector.memset(eps[:], EPS)
    for t in range(4):
        # per-tile stat chain on [128,1] slices: tile t's transposes start
        # without waiting for tiles t+1..3 (shrinks the PE-idle window at
        # every LN joint below the HAM re-throttle threshold)
        sl = slice(t, t + 1)
        nc.vector.reduce_sum(ssum[:, sl], x_ap[:, t, :], axis=mybir.AxisListType.X)
        sc = sb.tile([128, 1024], F32, tag="ln_sc")
        nc.scalar.activation(sc[:], x_ap[:, t, :], AF.Square,
                             accum_out=ssq[:, sl])
        nc.vector.tensor_scalar_mul(m[:, sl], ssum[:, sl], 1.0 / D)
        nc.vector.tensor_scalar_mul(var[:, sl], ssq[:, sl], 1.0 / D)
        nc.vector.tensor_mul(mm[:, sl], m[:, sl], m[:, sl])
        nc.vector.tensor_sub(var[:, sl], var[:, sl], mm[:, sl])
        nc.scalar.activation(std[:, sl], var[:, sl], AF.Sqrt, bias=eps[:])
        nc.vector.reciprocal(rstd[:, sl], std[:, sl])
        nc.vector.tensor_mul(nmr[:, sl], m[:, sl], rstd[:, sl])
        nc.vector.tensor_scalar_mul(nmr[:, sl], nmr[:, sl], -1.0)
        xn = sb.tile([128, 1024], BF16, tag="ln_xn")
        nc.vector.tensor_scalar(xn[:], x_ap[:, t, :], rstd[:, sl],
                                nmr[:, sl], OP.mult, OP.add)
        if DMA_TRANSPOSE:
            for kk in range(8):
                nc.sync.dma_start_transpose(
                    xn_fm[:, kk, t * 128:(t + 1) * 128],
                    xn[:, kk * 128:(kk + 1) * 128])
        else:
            ptr = psT.tile([128, 8, 128], BF16, tag="tr")
            for kk in range(8):
                nc.tensor.transpose(ptr[:, kk, :], xn[:, kk * 128:(kk + 1) * 128], ident[:])
            eng = nc.vector if (eng_evac + t) % 2 == 0 else nc.scalar
            if eng is nc.vector:
                nc.vector.tensor_copy(xn_fm[:, :, t * 128:(t + 1) * 128], ptr[:])
            else:
                nc.scalar.copy(xn_fm[:, :, t * 128:(t + 1) * 128], ptr[:])


def build(n_layers=L, dbg=False, no_cc=False, stages=99):
    nc = bacc.Bacc("TRN2", target_bir_lowering=False, debug=False,
                   num_devices=N_CORES)

    x0_h = nc.dram_tensor("x0", [TOK, D], F32, kind="ExternalInput")
    # pre-tiled weights: [chunks, 128(p), 8(kk), 512(c)] per layer
    wqkv_h = nc.dram_tensor("wqkv", [n_layers, 6, 128, 8, 512], BF16, kind="ExternalInput")
    wp_h = nc.dram_tensor("wp", [n_layers, 2, 128, 8, 512], BF16, kind="ExternalInput")
    w1_h = nc.dram_tensor("w1", [n_layers, 8, 128, 8, 512], BF16, kind="ExternalInput")
    w2_h = nc.dram_tensor("w2", [n_layers, 2, 8, 128, 4, 512], BF16, kind="ExternalInput")
    embT_h = nc.dram_tensor("embT", [NVC, 128, 8, 512], BF16, kind="ExternalInput")
    msk_h = nc.dram_tensor("msk", [2, 128, 128], BF16, kind="ExternalInput")
    ident_h = nc.dram_tensor("identin", [128, 128], BF16, kind="ExternalInput")
    ones64_h = nc.dram_tensor("ones64", [1, 64], BF16, kind="ExternalInput")
    out_h = nc.dram_tensor("out", [N_CORES * TOK, VP], BF16, kind="ExternalOutput")

    dbg_outs = {}

    def dbg_dump(name, ap, shape, rearr=None):
        if not dbg:
            return
        t = nc.dram_tensor(f"dbg_{name}", list(shape), ap.dtype, kind="ExternalOutput")
        dst = t.ap() if rearr is None else t.ap().rearrange(rearr)
        nc.sync.dma_start(dst, ap)
        dbg_outs[name] = shape

    ag_in = [nc.dram_tensor(f"agin{l}", [128, 4096], BF16, kind="Internal")
             for l in range(n_layers)]
    ag_out = [nc.dram_tensor(f"agout{l}", [256, 4096], BF16, kind="Internal")
              for l in range(n_layers)]
    # final AG split into token halves: [128, 8(kk) x 256(tok-half)]
    agf_in = [nc.dram_tensor(f"agfin{h}", [128, 2048], BF16, kind="Internal")
              for h in range(2)]
    agf_out = [nc.dram_tensor(f"agfout{h}", [N_CORES * 128, 2048], BF16,
                              kind="Internal", addr_space="Shared")
               for h in range(2)]

    with tile.TileContext(nc) as tc:
      with tc.tile_pool(name="const", bufs=1) as constp, \
           tc.tile_pool(name="xres", bufs=1) as xresp:
        with tc.tile_pool(name="stat", bufs=2) as stat, \
             tc.tile_pool(name="sb", bufs=2) as sb, \
             tc.tile_pool(name="act", bufs=1) as actp, \
             tc.tile_pool(name="wch", bufs=6) as wch, \
             tc.tile_pool(name="pp", bufs=8) as pp, \
             tc.tile_pool(name="psA", bufs=4, space="PSUM") as psA, \
             tc.tile_pool(name="psB", bufs=3, space="PSUM") as psB, \
             tc.tile_pool(name="psT", bufs=1, space="PSUM") as psT:

            ident = constp.tile([128, 128], BF16)
            nc.sync.dma_start(ident[:], ident_h[:])
            msk = constp.tile([128, 2, 128], BF16)
            nc.sync.dma_start(msk[:], msk_h.ap().rearrange("b p q -> p b q"))
            ones64 = constp.tile([1, 64], BF16)
            nc.sync.dma_start(ones64[:], ones64_h[:])

            # residual stream, token-major fp32 [part, tok-tile, D]
            x = xresp.tile([128, 4, D], F32)
            nc.sync.dma_start(x[:], x0_h.ap().rearrange("(t p) d -> p t d", p=128))

            for l in range(n_layers):
                # ---- LN1 -> xn_fm (feature-major bf16), kick pair AllGather
                xn_fm = actp.tile([128, 8, TOK], BF16, tag="xn_fm")
                _ln_tm(nc, sb, stat, psT, x, xn_fm, ident, eng_evac=0)
                nc.sync.dma_start(
                    ag_in[l].ap().rearrange("p (kk t) -> p kk t", kk=8), xn_fm[:])
                if no_cc:
                    nc.sync.dma_start(ag_out[l][0:128, :], ag_in[l][:])
                    nc.sync.dma_start(ag_out[l][128:256, :], ag_in[l][:])
                else:
                    nc.gpsimd.collective_compute(
                        "AllGather", OP.bypass, replica_groups=PAIRS,
                        ins=[ag_in[l][:]], outs=[ag_out[l][:]])
                if l == 0:
                    dbg_dump("xn_fm0", xn_fm[:], [128, 8, TOK])

                # ---- Q^T (feature-major) from local xn_fm (covers the AG)
                if stages < 3:
                    continue
                q_fm = actp.tile([128, 8, TOK], BF16, tag="q_fm")
                for ch in range(2):
                    wt = wch.tile([128, 8, 512], BF16, tag="w", name=f"wq{l}_{ch}")
                    nc.sync.dma_start(wt[:], wqkv_h[l, ch])
                    for mi in range(4):
                        ps = psA.tile([128, TOK], F32, tag="mm")
                        for kk in range(8):
                            nc.tensor.matmul(ps[:], wt[:, kk, mi * 128:(mi + 1) * 128],
                                             xn_fm[:, kk, :], start=(kk == 0), stop=(kk == 7))
                        nc.scalar.copy(q_fm[:, ch * 4 + mi, :], ps[:])

                # ---- K/V for LOCAL tokens (bank 0) — also covers the AG
                k_all = actp.tile([128, 16, TOK], BF16, tag="k_all")
                wkt = {}
                for ch in range(2):
                    wt = wch.tile([128, 8, 512], BF16, tag="w", name=f"wk{l}_{ch}")
                    nc.sync.dma_start(wt[:], wqkv_h[l, 2 + ch])
                    wkt[ch] = wt
                    for mi in range(4):
                        ps = psA.tile([128, TOK], F32, tag="mm")
                        for kk in range(8):
                            nc.tensor.matmul(
                                ps[:], wt[:, kk, mi * 128:(mi + 1) * 128],
                                xn_fm[:, kk, :], start=(kk == 0), stop=(kk == 7))
                        nc.vector.tensor_copy(k_all[:, ch * 4 + mi, :], ps[:])

                v_all = actp.tile([128, 8, H, HD + 1], BF16, tag="v_all")
                nc.vector.memset(v_all[:, :, :, HD:HD + 1], 1.0)
                wvt = {}
                for ch in range(2):
                    wt = wch.tile([128, 8, 512], BF16, tag="w", name=f"wv{l}_{ch}")
                    nc.sync.dma_start(wt[:], wqkv_h[l, 4 + ch])
                    wvt[ch] = wt
                    for t in range(4):
                        ps = psA.tile([128, TOK], F32, tag="mm")
                        for kk in range(8):
                            nc.tensor.matmul(
                                ps[:], xn_fm[:, kk, t * 128:(t + 1) * 128],
                                wt[:, kk, :], start=(kk == 0), stop=(kk == 7))
                        nc.vector.tensor_copy(
                            v_all[:, t, ch * 8:(ch + 1) * 8, 0:HD],
                            ps[:].rearrange("p (h d) -> p h d", h=8))

                # ---- gather pair xn; recover remote half bit-exactly as
                # (bank0 + bank1) - xn_local (fp32 intermediate => exact)
                if stages < 4:
                    continue
                xn_rem = actp.tile([128, 8, TOK], BF16, tag="xn_rem")
                for kk in range(8):
                    xb0 = sb.tile([128, TOK], BF16, tag="xb0")
                    xb1 = sb.tile([128, TOK], BF16, tag="xb1")
                    nc.sync.dma_start(xb0[:], ag_out[l][0:128, kk * TOK:(kk + 1) * TOK])
                    nc.sync.dma_start(xb1[:], ag_out[l][128:256, kk * TOK:(kk + 1) * TOK])
                    tmp = sb.tile([128, TOK], F32, tag="tmp32")
                    nc.vector.tensor_add(tmp[:], xb0[:], xb1[:])
                    nc.vector.tensor_sub(xn_rem[:, kk, :], tmp[:], xn_fm[:, kk, :])

                # ---- remote K/V and attention, interleaved in groups of two
                # head pairs: the scalar-bound exp stream of one group
                # overlaps the PE-bound remote-K/V matmuls of the next
                if stages < 5:
                    continue
                o_fm = actp.tile([128, 8, TOK], BF16, tag="o_fm")

                def attn_pair(hp):
                    he, ho = 2 * hp, 2 * hp + 1
                    ave = psB.tile([P, TOK], F32, tag="acc", name=f"av{l}_{he}")
                    avo = psB.tile([P, TOK], F32, tag="acc", name=f"av{l}_{ho}")
                    for b in range(2):
                        for i in range(4):
                            n = TOK - 128 * i
                            spe = psA.tile([128, n], F32, tag="mm")
                            spo = psA.tile([128, n], F32, tag="mm")
                            nc.tensor.matmul(
                                spe[:], k_all[0:64, b * 8 + hp, i * 128:(i + 1) * 128],
                                q_fm[0:64, hp, 128 * i:TOK], start=True, stop=True)
                            nc.tensor.matmul(
                                spo[:], k_all[64:128, b * 8 + hp, i * 128:(i + 1) * 128],
                                q_fm[64:128, hp, 128 * i:TOK], start=True, stop=True)
                            pte = pp.tile([128, n], BF16, tag="p")
                            pto = pp.tile([128, n], BF16, tag="p")
                            nc.scalar.activation(pte[:], spe[:], AF.Exp)
                            nc.scalar.activation(pto[:], spo[:], AF.Exp)
                            # causal mask on the diagonal 128-col block
                            nc.vector.tensor_mul(pte[:, 0:128], pte[:, 0:128], msk[:, b, :])
                            nc.vector.tensor_mul(pto[:, 0:128], pto[:, 0:128], msk[:, b, :])
                            nc.tensor.matmul(
                                ave[0:HD + 1, 128 * i:TOK],
                                v_all[:, b * 4 + i, he, :], pte[:],
                                start=(b == 0 and i == 0), stop=(b == 1 and i == 3))
                            nc.tensor.matmul(
                                avo[0:HD + 1, 128 * i:TOK],
                                v_all[:, b * 4 + i, ho, :], pto[:],
                                start=(b == 0 and i == 0), stop=(b == 1 and i == 3))
                    # denominators -> broadcast (ones64 MMs) -> fast approx
                    # reciprocal (~18 bits, plenty vs bf16 downstream)
                    den = sb.tile([1, 2, TOK], BF16, tag="den")
                    nc.vector.tensor_copy(den[0:1, 0, :], ave[HD:HD + 1, :])
                    nc.vector.tensor_copy(den[0:1, 1, :], avo[HD:HD + 1, :])
                    bp = psA.tile([128, TOK], F32, tag="mm")
                    nc.tensor.matmul(bp[0:64, :], ones64[:], den[0:1, 0, :],
                                     start=True, stop=True)
                    nc.tensor.matmul(bp[64:128, :], ones64[:], den[0:1, 1, :],
                                     start=True, stop=True)
                    rb = sb.tile([128, TOK], F32, tag="rb")
                    nc.vector.reciprocal_approx_fast(rb[:], bp[:])
                    nc.vector.tensor_tensor(o_fm[0:64, hp, :], ave[0:HD, :],
                                            rb[0:64, :], OP.mult)
                    nc.vector.tensor_tensor(o_fm[64:128, hp, :], avo[0:HD, :],
                                            rb[64:128, :], OP.mult)

                # remote K/V first (dense PE stream), attention after: the PE
                # queue is strict FIFO, so attention's exp-gated AV matmuls
                # must not sit in front of independent dense work
                for ch in range(2):
                    for mi in range(4):
                        ps = psA.tile([128, TOK], F32, tag="mm")
                        for kk in range(8):
                            nc.tensor.matmul(
                                ps[:], wkt[ch][:, kk, mi * 128:(mi + 1) * 128],
                                xn_rem[:, kk, :], start=(kk == 0), stop=(kk == 7))
                        nc.vector.tensor_copy(k_all[:, 8 + ch * 4 + mi, :], ps[:])
                for ch in range(2):
                    for t in range(4):
                        ps = psA.tile([128, TOK], F32, tag="mm")
                        for kk in range(8):
                            nc.tensor.matmul(
                                ps[:], xn_rem[:, kk, t * 128:(t + 1) * 128],
                                wvt[ch][:, kk, :], start=(kk == 0), stop=(kk == 7))
                        nc.vector.tensor_copy(
                            v_all[:, 4 + t, ch * 8:(ch + 1) * 8, 0:HD],
                            ps[:].rearrange("p (h d) -> p h d", h=8))
                for hp in range(H // 2):
                    attn_pair(hp)
                if l == 0:
                    dbg_dump("k_all0", k_all[:], [128, 16, TOK])
                    dbg_dump("v_all0", v_all[:], [128, 8, H, HD + 1])
                    dbg_dump("o_fm0", o_fm[:], [128, 8, TOK])

                # ---- projection (token-major) + residual
                if stages < 7:
                    continue
                for ch in range(2):
                    wt = wch.tile([128, 8, 512], BF16, tag="w", name=f"wpj{l}_{ch}")
                    nc.sync.dma_start(wt[:], wp_h[l, ch])
                    for t in range(4):
                        ps = psA.tile([128, 512], F32, tag="mm")
                        for kk in range(8):
                            nc.tensor.matmul(
                                ps[:], o_fm[:, kk, t * 128:(t + 1) * 128],
                                wt[:, kk, :], start=(kk == 0), stop=(kk == 7))
                        nc.vector.tensor_add(x[:, t, ch * 512:(ch + 1) * 512],
                                             x[:, t, ch * 512:(ch + 1) * 512], ps[:])
                if l == 0:
                    dbg_dump("xattn0", x[:], [128, 4, D])

                # ---- LN2 -> xn2_fm
                if stages < 8:
                    continue
                xn2_fm = actp.tile([128, 8, TOK], BF16, tag="xn2_fm")
                _ln_tm(nc, sb, stat, psT, x, xn2_fm, ident, eng_evac=1)

                # ---- FFN: ff1 full-token, ff2 in token halves
                h_sb = actp.tile([128, 32, TOK], BF16, tag="h_sb")
                for mc in range(8):
                    wt = wch.tile([128, 8, 512], BF16, tag="w", name=f"w1_{l}_{mc}")
                    nc.sync.dma_start(wt[:], w1_h[l, mc])
                    for mi in range(4):
                        ps = psA.tile([128, TOK], F32, tag="mm")
                        for kk in range(8):
                            nc.tensor.matmul(
                                ps[:], wt[:, kk, mi * 128:(mi + 1) * 128],
                                xn2_fm[:, kk, :], start=(kk == 0), stop=(kk == 7))
                        nc.scalar.activation(h_sb[:, mc * 4 + mi, :], ps[:], AF.Gelu)
                for half in range(2):
                    for nch in range(2):
                        acc = [psB.tile([128, 512], F32, tag="acc",
                                        name=f"acc{l}_{half}_{nch}_{a}") for a in range(2)]
                        for kkc in range(8):
                            w2t = wch.tile([128, 4, 512], BF16, tag="w",
                                           name=f"w2_{l}_{half}_{nch}_{kkc}")
                            nc.sync.dma_start(w2t[:], w2_h[l, nch, kkc])
                            for kki in range(4):
                                for mi in range(2):
                                    nc.tensor.matmul(
                                        acc[mi][:],
                                        h_sb[:, kkc * 4 + kki,
                                             half * 256 + mi * 128:half * 256 + (mi + 1) * 128],
                                        w2t[:, kki, :],
                                        start=(kkc == 0 and kki == 0),
                                        stop=(kkc == 7 and kki == 3))
                        for mi in range(2):
                            t = half * 2 + mi
                            nc.vector.tensor_add(x[:, t, nch * 512:(nch + 1) * 512],
                                                 x[:, t, nch * 512:(nch + 1) * 512],
                                                 acc[mi][:])
                if l == 0:
                    dbg_dump("xlayer0", x[:], [128, 4, D])

            if stages < 9:
                # early-exit build for bisection: dump residual so work isn't DCE'd
                xdump = nc.dram_tensor("xdump", [128, 4, D], F32, kind="ExternalOutput")
                nc.sync.dma_start(xdump.ap(), x[:])

        # ---- final LN + LM head phase (separate pools; trunk SBUF released)
        with tc.tile_pool(name="stat2", bufs=2) as stat2, \
             tc.tile_pool(name="sb2", bufs=2) as sb2, \
             tc.tile_pool(name="hd", bufs=1) as hd, \
             tc.tile_pool(name="emb", bufs=3) as epool, \
             tc.tile_pool(name="hout", bufs=4) as hout, \
             tc.tile_pool(name="psT2", bufs=1, space="PSUM") as psT2, \
             tc.tile_pool(name="psH", bufs=4, space="PSUM") as psH:
            if stages >= 9:
                xnf_fm = hd.tile([128, 8, TOK], BF16)
                _ln_tm(nc, sb2, stat2, psT2, x, xnf_fm, ident, eng_evac=0)
                # world AllGather split into token halves so head MMs for the
                # first half start while the second half is still in flight
                for hh in range(2):
                    nc.sync.dma_start(
                        agf_in[hh].ap().rearrange("p (kk t) -> p kk t", kk=8),
                        xnf_fm[:, :, hh * 256:(hh + 1) * 256])
                    if no_cc:
                        for r_ in range(N_CORES):
                            nc.sync.dma_start(
                                agf_out[hh][r_ * 128:(r_ + 1) * 128, :], agf_in[hh][:])
                    else:
                        nc.gpsimd.collective_compute(
                            "AllGather", OP.bypass, replica_groups=WORLD,
                            ins=[agf_in[hh][:]], outs=[agf_out[hh][:]])
                if dbg:
                    dbg_dump("xnf_fm", xnf_fm[:], [128, 8, TOK])
                xn_all = hd.tile([128, 64, TOK], BF16)
                for hh in range(2):
                    for r_ in range(8):
                        nc.sync.dma_start(
                            xn_all[:, r_ * 8:(r_ + 1) * 8, hh * 256:(hh + 1) * 256],
                            agf_out[hh][r_ * 128:(r_ + 1) * 128, :].rearrange(
                                "p (kk t) -> p kk t", kk=8))
                nchunks = [(i * 512, min(512, VP - i * 512)) for i in range(NVC)]
                for ni, (n0, nsz) in enumerate(nchunks):
                    et = epool.tile([128, 8, 512], BF16, tag="emb")
                    nc.sync.dma_start(et[:], embT_h[ni])
                    # token half 0 (tiles 0,1 of every rank) first: available
                    # as soon as the first half-AG lands
                    for mi in ([m for m in range(32) if m % 4 < 2]
                               + [m for m in range(32) if m % 4 >= 2]):
                        r, t = mi // 4, mi % 4
                        ps = psH.tile([128, nsz], F32, tag="h")
                        for kk in range(8):
                            nc.tensor.matmul(
                                ps[:], xn_all[:, r * 8 + kk, t * 128:(t + 1) * 128],
                                et[:, kk, 0:nsz],
                                start=(kk == 0), stop=(kk == 7))
                        osb = hout.tile([128, nsz], BF16, tag="o")
                        if mi % 2 == 0:
                            nc.vector.tensor_copy(osb[:], ps[:])
                        else:
                            nc.scalar.copy(osb[:], ps[:])
                        nc.sync.dma_start(out_h[mi * 128:(mi + 1) * 128, n0:n0 + nsz], osb[:])

    nc.compile()
    return nc, dbg_outs


def _fm_tile_w(w):
    """[1024, nch*512] -> [nch, 128, 8, 512]; tile[j,p,kk,c] = w[kk*128+p, j*512+c]."""
    din, dout = w.shape
    nch = dout // 512
    r = w.reshape(8, 128, nch, 512)
    return np.ascontiguousarray(r.transpose(2, 1, 0, 3))


def prepare_inputs(idx, tok_emb, pos_emb, qkv_w, qkv_b, proj_w, proj_b,
                   ff1_w, ff1_b, ff2_w, ff2_b, ln1_s, ln1_b, ln2_s, ln2_b,
                   lnf_s, lnf_b, n_layers=L):
    """Host-side sharding/folding. Returns per-core in_maps."""
    bf = ml_dtypes.bfloat16
    for name, v in (("qkv_b", qkv_b), ("proj_b", proj_b), ("ff1_b", ff1_b),
                    ("ff2_b", ff2_b), ("ln1_b", ln1_b), ("ln2_b", ln2_b),
                    ("lnf_b", lnf_b)):
        assert np.allclose(np.asarray(v), 0.0), f"nonzero {name} not supported"

    idx = np.asarray(idx)
    tok_emb = np.asarray(tok_emb, np.float32)
    pos_emb = np.asarray(pos_emb, np.float32)
    scale = 1.0 / np.sqrt(HD)

    # fold LN scales + attention scale into weights (exact)
    wqkv = (np.asarray(qkv_w[:n_layers], np.float32)
            * np.asarray(ln1_s[:n_layers], np.float32)[:, :, None]).copy()
    wqkv[:, :, :D] *= scale
    w1 = (np.asarray(ff1_w[:n_layers], np.float32)
          * np.asarray(ln2_s[:n_layers], np.float32)[:, :, None])
    wp = np.asarray(proj_w[:n_layers], np.float32)
    w2 = np.asarray(ff2_w[:n_layers], np.float32)
    embT_full = (tok_emb * np.asarray(lnf_s, np.float32)[None, :]).T  # [D, V]
    embT_pad = np.zeros((D, N_CORES * VP), np.float32)
    embT_pad[:, :V] = embT_full

    # pre-tiled weight arrays (contiguous 1MB DMA bursts on device)
    wqkv_t = np.stack([_fm_tile_w(wqkv[l]) for l in range(n_layers)]).astype(bf)
    wp_t = np.stack([_fm_tile_w(wp[l]) for l in range(n_layers)]).astype(bf)
    w1_t = np.stack([_fm_tile_w(w1[l]) for l in range(n_layers)]).astype(bf)
    # w2: [4096, 1024] -> [2(nch), 8(kkc), 128(p), 4(kki), 512(c)]
    w2_t = np.stack([
        np.ascontiguousarray(
            w2[l].reshape(8, 4, 128, 2, 512).transpose(3, 0, 2, 1, 4))
        for l in range(n_layers)]).astype(bf)

    ident = np.eye(128, dtype=bf)
    ones64 = np.ones((1, 64), bf)

    tri = np.tril(np.ones((128, 128), np.float32)).T  # [kt, q] valid kt<=q
    # core-relative banks: slot0 = local diagonal (triangular for both
    # ranks); slot1 = remote diagonal (all-masked for r=0, visible for r=1)
    msk_r = [np.zeros((2, 128, 128), np.float32) for _ in range(2)]
    msk_r[0][0] = tri
    msk_r[0][1] = 0.0
    msk_r[1][0] = tri
    msk_r[1][1] = 1.0

    in_maps = []
    for c in range(N_CORES):
        b, r = c // 2, c % 2
        pos = positions_for_rank(r)
        x0 = tok_emb[idx[b, pos]] + pos_emb[pos]
        # per-core vocab slice, padded to 13*512 cols for uniform DMA
        esl = np.zeros((D, NVC * 512), np.float32)
        esl[:, :VP] = embT_pad[:, c * VP:(c + 1) * VP]
        embT_tiles = np.ascontiguousarray(
            esl.reshape(8, 128, NVC, 512).transpose(2, 1, 0, 3)).astype(bf)
        in_maps.append({
            "x0": np.ascontiguousarray(x0, np.float32),
            "wqkv": wqkv_t, "wp": wp_t, "w1": w1_t, "w2": w2_t,
            "embT": embT_tiles,
            "msk": msk_r[r].astype(bf),
            "identin": ident,
            "ones64": ones64,
        })
    return in_maps


def assemble_output(results):
    """Per-core [4096, VP] bf16 -> full logits [B, T, V] f32."""
    logits = np.empty((B, T, V), np.float32)
    pos_r = [positions_for_rank(0), positions_for_rank(1)]
    for c in range(N_CORES):
        out_c = np.asarray(results[c]["out"], np.float32)  # [4096, VP]
        v0 = c * VP
        ncols = min(VP, V - v0)
        if ncols <= 0:
            continue
        for r in range(N_CORES):
            bb, rr = r // 2, r % 2
            logits[bb, pos_r[rr], v0:v0 + ncols] = \
                out_c[r * TOK:(r + 1) * TOK, :ncols]
    return logits


_NC_CACHE = {}


def _get_nc(n_layers=L, dbg=False):
    key = (n_layers, dbg)
    if key not in _NC_CACHE:
        _NC_CACHE[key] = build(n_layers=n_layers, dbg=dbg)
    return _NC_CACHE[key]


def kernel(**inputs):
    in_maps = prepare_inputs(**inputs)
    nc, _ = _get_nc()
    res = run_bass_kernel_spmd(nc, in_maps, core_ids=list(range(N_CORES)))
    return assemble_output(res.results)
